# revision 14
# baseline (speedup 1.0000x reference)
"""Trainium2 Bass kernel for nn_LlamaAttention_48816598286577.

Llama attention with block-streaming sparse mask (sink=1 block, local
window=8 blocks, BLOCK=128), B=1 S=2048 H=4096, 32 q heads / 8 kv heads,
head_dim 128, non-interleaved RoPE.

Sharding: tensor-parallel over heads across 8 cores (4 q heads + 1 kv
head per core). All matmul operands are bf16 (f32r measured ~2x slower
per row and ~4x slower LDWEIGHTS on HW); accumulation is f32 in PSUM.

Schedule: the S=2048 sequence is processed in 4 projection quarters.
After each quarter's QKV+RoPE, the two 256-query attention chunks it
unlocks run immediately, each followed by its bf16 AllGather; o_proj
for chunk i runs two chunks later, so every AllGather hides under
attention + o_proj + next-quarter PE work. DMA descriptor generation is
spread across the Sync/Scalar/Vector/GpSimd queues to avoid the
in-order SP queue serializing issue (565ns each).
"""

import functools
import numpy as np
import ml_dtypes

import concourse.bass as bass
import concourse.mybir as mybir
import concourse.tile as tile
from concourse import bacc
from concourse.bass_utils import run_bass_kernel_spmd

# problem constants (hardcoded per contract)
B, S, H = 1, 2048, 4096
NQ, NKV, HD = 32, 8, 128
BLOCK = 128
NBLK = S // BLOCK          # 16
SINK_BLOCKS = 1
LOCAL_BLOCKS = 8
ROPE_BASE = 10000.0
N_CORES = 8
HQ = NQ // N_CORES         # 4 q heads per core
DQ = HQ * HD               # 512 q columns per core
SCALE = 1.0 / float(np.sqrt(HD))

KC = H // 128              # 32 contraction chunks for projections
NQUART = 4                 # S split into 4 quarters of 512 for projections
QW = S // NQUART           # 512
NCH = NBLK // 2            # 8 attention chunks of 256 queries

F32 = mybir.dt.float32
BF16 = mybir.dt.bfloat16

# Opt-in profiling plumbing (off by default; harness never touches these).
TRACE = False
TRACE_KW: dict = {}
LAST_RESULTS = None


def _pair_js(i: int) -> list[int]:
    """Key blocks contributing to query pair i (blocks 2i, 2i+1)."""
    return sorted(set([0]) | set(range(max(0, 2 * i - 7), 2 * i + 2)))


def build_nc(rs: bool = True, repeat: int = 1):
    nc = bacc.Bacc(
        "TRN2", target_bir_lowering=False, debug=False, num_devices=N_CORES
    )
    hidT = nc.dram_tensor("hidT", [H, S], BF16, kind="ExternalInput").ap()
    wq = nc.dram_tensor("wq", [H, DQ], BF16, kind="ExternalInput").ap()
    wk = nc.dram_tensor("wk", [H, HD], BF16, kind="ExternalInput").ap()
    wv = nc.dram_tensor("wv", [H, HD], BF16, kind="ExternalInput").ap()
    wo = nc.dram_tensor("wo", [H, DQ], BF16, kind="ExternalInput").ap()
    cosF = nc.dram_tensor("cosF", [128, S], F32, kind="ExternalInput").ap()
    sinS = nc.dram_tensor("sinS", [128, S], F32, kind="ExternalInput").ap()
    tri = nc.dram_tensor("tri", [128, 128], BF16, kind="ExternalInput").ap()
    eye = nc.dram_tensor("eye", [128, 128], F32, kind="ExternalInput").ap()
    onescol = nc.dram_tensor("onescol", [128, 1], BF16, kind="ExternalInput").ap()
    maskA = nc.dram_tensor("maskA", [128, 256], BF16, kind="ExternalInput").ap()
    out = nc.dram_tensor("out", [S, DQ], F32, kind="ExternalOutput").ap()

    with tile.TileContext(nc) as tc:
      for rep in range(repeat):
        with (
            tc.tile_pool(name=f"persist{rep}", bufs=1) as pp,
            tc.tile_pool(name=f"dram{rep}", bufs=1, space="DRAM") as dramp,
            tc.tile_pool(name=f"sp{rep}", bufs=1) as sp,
            tc.tile_pool(name=f"stream{rep}", bufs=3) as stp,
            tc.tile_pool(name=f"e_sb{rep}", bufs=3) as ep,
            tc.tile_pool(name=f"att_sb{rep}", bufs=2) as asb,
            tc.tile_pool(name=f"ev_sb{rep}", bufs=3) as evp,
        ):
            qTr = [
                [
                    pp.tile([128, QW], BF16, tag=f"qTr{h}_{nq}", name=f"qTr{h}_{nq}")
                    for nq in range(NQUART)
                ]
                for h in range(HQ)
            ]
            kTr = [
                pp.tile([128, QW], BF16, tag=f"kTr{nq}", name=f"kTr{nq}")
                for nq in range(NQUART)
            ]
            vNat = [
                pp.tile([128, QW], BF16, tag=f"vNat{nq}", name=f"vNat{nq}")
                for nq in range(NQUART)
            ]
            tri_sb = pp.tile([128, 128], BF16, tag="tri", name="tri_sb")
            maskA_sb = pp.tile([128, 256], BF16, tag="maskA", name="maskA_sb")
            eye_sb = pp.tile([128, 128], F32, tag="eye", name="eye_sb")
            ones_sb = pp.tile([128, 1], BF16, tag="ones", name="ones_sb")
            nc.sync.dma_start(tri_sb[:], tri[:])
            nc.sync.dma_start(maskA_sb[:], maskA[:])
            nc.sync.dma_start(eye_sb[:], eye[:])
            nc.sync.dma_start(ones_sb[:], onescol[:])

            # CC-stream warmup collective, issued first on gpsimd so the
            # ~50us NRT stream init overlaps the first projection quarter.
            if rs:
                warm_in = dramp.tile([16, 16], BF16, tag="warm_in", name="warm_in")
                warm_out = dramp.tile(
                    [128, 16], BF16, tag="warm_out", name="warm_out",
                    addr_space="Shared",
                )
                warm_src = pp.tile([16, 16], BF16, tag="warm_src", name="warm_src")
                nc.vector.memset(warm_src[:], 0.0)
                nc.gpsimd.dma_start(warm_in[:], warm_src[:])
                nc.gpsimd.collective_compute(
                    "AllGather",
                    mybir.AluOpType.bypass,
                    replica_groups=[list(range(N_CORES))],
                    ins=[warm_in.opt()],
                    outs=[warm_out.opt()],
                )

            # weight prefetch: qkv weights issue on SP in consumption order
            # (interleaved per chunk, pacing the quarter-0 matmuls); wo on
            # gpsimd (needed only from the second slot on).
            wo_t = []
            for c in range(KC):
                tw = sp.tile([128, DQ], BF16, tag=f"woc{c}", name=f"woc{c}")
                nc.gpsimd.dma_start(tw[:], wo[c * 128 : (c + 1) * 128, :])
                wo_t.append(tw)
            wq_t, wk_t, wv_t = [], [], []
            for c in range(KC):
                crow = slice(c * 128, (c + 1) * 128)
                tq = sp.tile([128, DQ], BF16, tag=f"wqc{c}", name=f"wqc{c}")
                tk = sp.tile([128, HD], BF16, tag=f"wkc{c}", name=f"wkc{c}")
                tv = sp.tile([128, HD], BF16, tag=f"wvc{c}", name=f"wvc{c}")
                nc.sync.dma_start(tq[:], wq[crow, :])
                nc.sync.dma_start(tk[:], wk[crow, :])
                nc.sync.dma_start(tv[:], wv[crow, :])
                wq_t.append(tq)
                wk_t.append(tk)
                wv_t.append(tv)

            vT = sp.tile([128, S], F32, tag="vT", name="vT")
            ag_ins = [
                dramp.tile([DQ, 256], BF16, tag=f"agin{c}", name=f"agin{c}")
                for c in range(NCH)
            ]
            ag_outs = [
                dramp.tile(
                    [H, 256], BF16, tag=f"agout{c}", name=f"agout{c}",
                    addr_space="Shared",
                )
                for c in range(NCH)
            ]

            def attn_chunk(i: int, apool):
                q0 = i * 256
                js = _pair_js(i)
                L = len(js)
                for h in range(HQ):
                    e_t = ep.tile([128, L * 256], BF16, tag="e", name="e_t")
                    oT = apool.tile([128, 256], F32, tag="oT", name="oT", bufs=2)
                    sm = apool.tile([1, 256], F32, tag="sm", name="sm", bufs=1)

                    spans = []
                    for j in js:
                        left = (j == 0) or (j <= 2 * i <= j + 7)
                        right = (j == 0) or (j <= 2 * i + 1 <= j + 7)
                        qs = q0 if left else q0 + 128
                        qe = q0 + 256 if right else q0 + 128
                        spans.append((qs, qe))

                    def score(idx: int):
                        j = js[idx]
                        qs, qe = spans[idx]
                        w = qe - qs
                        ecols = slice(idx * 256, idx * 256 + w)
                        s_ps = apool.tile(
                            [128, 256], F32, tag="sps", name="s_ps", bufs=3
                        )
                        kq_, kc_ = j // 4, (j % 4) * 128
                        qq_ = qs // QW
                        nc.tensor.matmul(
                            s_ps[:, 0:w],
                            kTr[kq_][:, kc_ : kc_ + 128],
                            qTr[h][qq_][:, qs - qq_ * QW : qe - qq_ * QW],
                            start=True,
                            stop=True,
                        )
                        nc.scalar.activation(
                            e_t[:, ecols],
                            s_ps[:, 0:w],
                            mybir.ActivationFunctionType.Exp,
                            scale=SCALE,
                        )
                        if j == 2 * i:
                            nc.vector.tensor_mul(
                                e_t[:, ecols], e_t[:, ecols], maskA_sb[:]
                            )
                        elif j == 2 * i + 1:
                            nc.vector.tensor_mul(
                                e_t[:, ecols], e_t[:, ecols], tri_sb[:]
                            )

                    def av(idx: int):
                        j = js[idx]
                        qs, qe = spans[idx]
                        w = qe - qs
                        ecols = slice(idx * 256, idx * 256 + w)
                        st, sp_ = (idx == 0), (idx == L - 1)
                        nc.tensor.matmul(
                            oT[:, qs - q0 : qe - q0],
                            vNat[j // 4][:, (j % 4) * 128 : (j % 4 + 1) * 128],
                            e_t[:, ecols],
                            start=st,
                            stop=sp_,
                        )
                        nc.tensor.matmul(
                            sm[:, qs - q0 : qe - q0],
                            ones_sb[:],
                            e_t[:, ecols],
                            start=st,
                            stop=sp_,
                        )

                    score(0)
                    if L > 1:
                        score(1)
                    for idx in range(L):
                        if idx + 2 < L:
                            score(idx + 2)
                        av(idx)

                    r_sb = asb.tile([1, 256], F32, tag="r", name="r_sb")
                    nc.vector.reciprocal_approx_fast(r_sb[:], sm[:])
                    rb = asb.tile([128, 256], F32, tag="rb", name="rb")
                    nc.gpsimd.partition_broadcast(rb[:], r_sb[:])
                    at_c = asb.tile([128, 256], BF16, tag=f"at{h}", name=f"at{h}")
                    nc.vector.tensor_mul(at_c[:], oT[:], rb[:])
                    nc.sync.dma_start(
                        ag_ins[i][h * 128 : (h + 1) * 128, :], at_c[:]
                    )

                if rs:
                    nc.gpsimd.collective_compute(
                        "AllGather",
                        mybir.AluOpType.bypass,
                        replica_groups=[list(range(N_CORES))],
                        ins=[ag_ins[i].opt()],
                        outs=[ag_outs[i].opt()],
                    )
                else:
                    nc.sync.dma_start(ag_outs[i][0:DQ, :], ag_ins[i][:])

            def oproj_chunk(i: int, oppool):
                q0 = i * 256
                ps01 = [
                    oppool.tile(
                        [128, 512], F32, tag=f"op{sb}", name=f"op{sb}", bufs=1
                    )
                    for sb in range(2)
                ]
                # ag loads + out writes issue on gpsimd: they depend on the
                # collective, and on the SP queue a hoisted one head-of-line
                # blocks the next quarter's hid stream behind the AllGather
                for c in range(KC):
                    ag_sb = evp.tile(
                        [128, 256], BF16, tag="ag_sb", name="ag_sb", bufs=6
                    )
                    nc.gpsimd.dma_start(
                        ag_sb[:], ag_outs[i][c * 128 : (c + 1) * 128, :]
                    )
                    for sb in range(2):
                        nc.tensor.matmul(
                            ps01[sb][:],
                            ag_sb[:, sb * 128 : (sb + 1) * 128],
                            wo_t[c][:],
                            start=(c == 0),
                            stop=(c == KC - 1),
                        )
                for sb in range(2):
                    ev = evp.tile([128, 512], F32, tag="ev", name="ev")
                    nc.scalar.copy(ev[:], ps01[sb][:])
                    nc.gpsimd.dma_start(
                        out[q0 + sb * 128 : q0 + (sb + 1) * 128, :], ev[:]
                    )

            for nq in range(NQUART):
                ncols = slice(nq * QW, (nq + 1) * QW)
                with tc.tile_pool(
                    name=f"qps{rep}_{nq}", bufs=1, space="PSUM"
                ) as qpool:
                    ps_q = [
                        qpool.tile([128, QW], F32, tag=f"psq{h}", name=f"psq{h}")
                        for h in range(HQ)
                    ]
                    ps_k = qpool.tile([128, QW], F32, tag="psk", name="ps_k")
                    ps_v = qpool.tile([128, QW], F32, tag="psv", name="ps_v")
                    # quarter 0's hid stream issues on ACT so it doesn't sit
                    # behind the 96 weight DMAs on the SP queue
                    hid_eng = nc.scalar if nq == 0 else nc.sync
                    for c in range(KC):
                        crow = slice(c * 128, (c + 1) * 128)
                        hid_c = stp.tile([128, QW], BF16, tag="hid", name="hid_c")
                        hid_eng.dma_start(hid_c[:], hidT[crow, ncols])
                        st, sp_ = (c == 0), (c == KC - 1)
                        for h in range(HQ):
                            nc.tensor.matmul(
                                ps_q[h][:],
                                wq_t[c][:, h * HD : (h + 1) * HD],
                                hid_c[:],
                                start=st,
                                stop=sp_,
                            )
                        nc.tensor.matmul(
                            ps_k[:], wk_t[c][:], hid_c[:], start=st, stop=sp_
                        )
                        nc.tensor.matmul(
                            ps_v[:], wv_t[c][:], hid_c[:], start=st, stop=sp_
                        )

                    cos_sb = stp.tile([128, QW], F32, tag="cos", name="cos_sb", bufs=2)
                    sin_sb = stp.tile([128, QW], F32, tag="sin", name="sin_sb", bufs=2)
                    nc.sync.dma_start(cos_sb[:], cosF[:, ncols])
                    nc.sync.dma_start(sin_sb[:], sinS[:, ncols])

                    # Evacuate PSUM on ACT and RoPE on DVE, grouped per
                    # tensor with q0 first then k: the first attention chunk
                    # needs qTr[0] and the fresh kTr earliest. swp DMAs issue
                    # on ACT right after the raw copy they read.
                    rope_list = [(ps_q[0], qTr[0][nq], "q0"), (ps_k, kTr[nq], "k")]
                    rope_list += [
                        (ps_q[h], qTr[h][nq], f"q{h}") for h in range(1, HQ)
                    ]
                    for ps_x, dstT, tag in rope_list:
                        raw = sp.tile([128, QW], F32, tag=f"raw{tag}", name=f"raw{tag}")
                        nc.scalar.copy(raw[:], ps_x[:])
                        swp = sp.tile([128, QW], F32, tag=f"swp{tag}", name=f"swp{tag}")
                        nc.scalar.dma_start(swp[0:64, :], raw[64:128, :])
                        nc.scalar.dma_start(swp[64:128, :], raw[0:64, :])
                        t1 = sp.tile([128, QW], F32, tag=f"t1{tag}", name=f"t1{tag}")
                        nc.vector.tensor_mul(t1[:], raw[:], cos_sb[:])
                        nc.vector.tensor_mul(swp[:], swp[:], sin_sb[:])
                        nc.vector.tensor_add(dstT[:], t1[:], swp[:])
                    nc.scalar.copy(vT[:, ncols], ps_v[:])

                    # V natural blocks for this quarter (4 transposes)
                    with tc.tile_pool(
                        name=f"trp{rep}_{nq}", bufs=2, space="PSUM"
                    ) as trpool:
                        for jb in range(nq * QW // 128, (nq + 1) * QW // 128):
                            bcols = slice(jb * 128, (jb + 1) * 128)
                            lcols = slice((jb % 4) * 128, (jb % 4 + 1) * 128)
                            tr = trpool.tile([128, 128], F32, tag="tr", name="tr")
                            nc.tensor.transpose(tr[:], vT[:, bcols], eye_sb[:])
                            nc.scalar.copy(vNat[nq][:, lcols], tr[:])

                # attention chunks unlocked by this quarter; o_proj lags by
                # 2 chunks so each AllGather hides under subsequent PE work.
                # The last slot runs chunk 7 before 6 so AG7 is covered by
                # attn(6) + o_proj work instead of sticking out as a tail.
                # Each slot starts with the lagged o_proj (depends only on an
                # old AllGather), covering this quarter's RoPE latency before
                # attention needs the fresh qTr/kTr.
                ca, cb = (2 * nq, 2 * nq + 1) if nq < 3 else (7, 6)
                with tc.tile_pool(
                    name=f"aps{rep}_{nq}", bufs=1, space="PSUM"
                ) as apool:
                    if 2 * nq - 2 >= 0:
                        with tc.tile_pool(
                            name=f"ops{rep}_{nq}a", bufs=1, space="PSUM"
                        ) as oppool:
                            oproj_chunk(2 * nq - 2, oppool)
                    attn_chunk(ca, apool)
                    if 2 * nq - 1 >= 0:
                        with tc.tile_pool(
                            name=f"ops{rep}_{nq}b", bufs=1, space="PSUM"
                        ) as oppool:
                            oproj_chunk(2 * nq - 1, oppool)
                    attn_chunk(cb, apool)

            with tc.tile_pool(
                name=f"ops{rep}_t7", bufs=1, space="PSUM"
            ) as oppool:
                oproj_chunk(NCH - 1, oppool)
            with tc.tile_pool(
                name=f"ops{rep}_t6", bufs=1, space="PSUM"
            ) as oppool:
                oproj_chunk(NCH - 2, oppool)
    nc.compile()
    return nc


@functools.lru_cache(maxsize=1)
def _cached_nc():
    return build_nc(rs=True)


def _tables():
    pos = np.arange(S, dtype=np.float64)
    inv = 1.0 / (ROPE_BASE ** (np.arange(0, HD, 2, dtype=np.float64) / HD))  # [64]
    f = inv[:, None] * pos[None, :]                   # [64, S]
    cos = np.cos(f).astype(np.float32)
    sin = np.sin(f).astype(np.float32)
    cosF = np.concatenate([cos, cos], axis=0)         # [128, S]
    sinS = np.concatenate([-sin, sin], axis=0)        # [128, S]
    k_idx = np.arange(128)[:, None]
    q_idx = np.arange(128)[None, :]
    tri = (k_idx <= q_idx).astype(np.float32)         # [k, q] causal in-block
    eye = np.eye(128, dtype=np.float32)
    maskA = np.concatenate([tri, np.ones((128, 128), np.float32)], axis=1)
    return cosF, sinS, tri, eye, maskA


def _bf16(x: np.ndarray) -> np.ndarray:
    return np.ascontiguousarray(x).astype(ml_dtypes.bfloat16)


def kernel(hidden_states, wq, wk, wv, wo):
    nc = _cached_nc()
    hidT = _bf16(np.asarray(hidden_states, dtype=np.float32).reshape(S, H).T)
    cosF, sinS, tri, eye, maskA = _tables()
    in_maps = []
    for c in range(N_CORES):
        in_maps.append(
            {
                "hidT": hidT,
                "wq": _bf16(wq[:, c * DQ : (c + 1) * DQ]),
                "wk": _bf16(wk[:, c * HD : (c + 1) * HD]),
                "wv": _bf16(wv[:, c * HD : (c + 1) * HD]),
                "wo": _bf16(wo[:, c * DQ : (c + 1) * DQ]),
                "cosF": cosF,
                "sinS": sinS,
                "tri": _bf16(tri),
                "eye": eye,
                "onescol": np.ones((128, 1), dtype=ml_dtypes.bfloat16),
                "maskA": _bf16(maskA),
            }
        )
    kw = dict(trace=True, **TRACE_KW) if TRACE else {}
    res = run_bass_kernel_spmd(nc, in_maps, core_ids=list(range(N_CORES)), **kw)
    global LAST_RESULTS
    LAST_RESULTS = res
    full = np.concatenate(
        [res.results[r]["out"] for r in range(N_CORES)], axis=1
    )
    return full.reshape(B, S, H)


# revision 16
# speedup vs baseline: 1.0651x; 1.0651x over previous
"""Trainium2 Bass kernel for nn_LlamaAttention_48816598286577.

Llama attention with block-streaming sparse mask (sink=1 block, local
window=8 blocks, BLOCK=128), B=1 S=2048 H=4096, 32 q heads / 8 kv heads,
head_dim 128, non-interleaved RoPE.

Sharding: tensor-parallel over heads across 8 cores (4 q heads + 1 kv
head per core). All matmul operands are bf16 (f32r measured ~2x slower
per row and ~4x slower LDWEIGHTS on HW); accumulation is f32 in PSUM.

Schedule: the S=2048 sequence is processed in 4 projection quarters.
After each quarter's QKV+RoPE, the two 256-query attention chunks it
unlocks run immediately, each followed by its bf16 AllGather; o_proj
for chunk i runs two chunks later, so every AllGather hides under
attention + o_proj + next-quarter PE work. DMA descriptor generation is
spread across the Sync/Scalar/Vector/GpSimd queues to avoid the
in-order SP queue serializing issue (565ns each).
"""

import functools
import numpy as np
import ml_dtypes

import concourse.bass as bass
import concourse.mybir as mybir
import concourse.tile as tile
from concourse import bacc
from concourse.bass_utils import run_bass_kernel_spmd

# problem constants (hardcoded per contract)
B, S, H = 1, 2048, 4096
NQ, NKV, HD = 32, 8, 128
BLOCK = 128
NBLK = S // BLOCK          # 16
SINK_BLOCKS = 1
LOCAL_BLOCKS = 8
ROPE_BASE = 10000.0
N_CORES = 8
HQ = NQ // N_CORES         # 4 q heads per core
DQ = HQ * HD               # 512 q columns per core
SCALE = 1.0 / float(np.sqrt(HD))

KC = H // 128              # 32 contraction chunks for projections
NQUART = 4                 # S split into 4 quarters of 512 for projections
QW = S // NQUART           # 512
NCH = NBLK // 2            # 8 attention chunks of 256 queries

F32 = mybir.dt.float32
BF16 = mybir.dt.bfloat16

# Opt-in profiling plumbing (off by default; harness never touches these).
TRACE = False
TRACE_KW: dict = {}
LAST_RESULTS = None


def _pair_js(i: int) -> list[int]:
    """Key blocks contributing to query pair i (blocks 2i, 2i+1)."""
    return sorted(set([0]) | set(range(max(0, 2 * i - 7), 2 * i + 2)))


def build_nc(rs: bool = True, repeat: int = 1):
    nc = bacc.Bacc(
        "TRN2", target_bir_lowering=False, debug=False, num_devices=N_CORES
    )
    hidT = nc.dram_tensor("hidT", [H, S], BF16, kind="ExternalInput").ap()
    wq = nc.dram_tensor("wq", [H, DQ], BF16, kind="ExternalInput").ap()
    wk = nc.dram_tensor("wk", [H, HD], BF16, kind="ExternalInput").ap()
    wv = nc.dram_tensor("wv", [H, HD], BF16, kind="ExternalInput").ap()
    wo = nc.dram_tensor("wo", [H, DQ], BF16, kind="ExternalInput").ap()
    cosF = nc.dram_tensor("cosF", [128, S], F32, kind="ExternalInput").ap()
    sinS = nc.dram_tensor("sinS", [128, S], F32, kind="ExternalInput").ap()
    tri = nc.dram_tensor("tri", [128, 128], BF16, kind="ExternalInput").ap()
    eye = nc.dram_tensor("eye", [128, 128], F32, kind="ExternalInput").ap()
    onescol = nc.dram_tensor("onescol", [128, 1], BF16, kind="ExternalInput").ap()
    maskA = nc.dram_tensor("maskA", [128, 256], BF16, kind="ExternalInput").ap()
    out = nc.dram_tensor("out", [S, DQ], F32, kind="ExternalOutput").ap()

    with tile.TileContext(nc) as tc:
      for rep in range(repeat):
        with (
            tc.tile_pool(name=f"persist{rep}", bufs=1) as pp,
            tc.tile_pool(name=f"dram{rep}", bufs=1, space="DRAM") as dramp,
            tc.tile_pool(name=f"sp{rep}", bufs=1) as sp,
            tc.tile_pool(name=f"stream{rep}", bufs=3) as stp,
            tc.tile_pool(name=f"e_sb{rep}", bufs=3) as ep,
            tc.tile_pool(name=f"att_sb{rep}", bufs=2) as asb,
            tc.tile_pool(name=f"ev_sb{rep}", bufs=3) as evp,
        ):
            qTr = [
                [
                    pp.tile([128, QW], BF16, tag=f"qTr{h}_{nq}", name=f"qTr{h}_{nq}")
                    for nq in range(NQUART)
                ]
                for h in range(HQ)
            ]
            kTr = [
                pp.tile([128, QW], BF16, tag=f"kTr{nq}", name=f"kTr{nq}")
                for nq in range(NQUART)
            ]
            vNat = [
                pp.tile([128, QW], BF16, tag=f"vNat{nq}", name=f"vNat{nq}")
                for nq in range(NQUART)
            ]
            tri_sb = pp.tile([128, 128], BF16, tag="tri", name="tri_sb")
            maskA_sb = pp.tile([128, 256], BF16, tag="maskA", name="maskA_sb")
            eye_sb = pp.tile([128, 128], F32, tag="eye", name="eye_sb")
            ones_sb = pp.tile([128, 1], BF16, tag="ones", name="ones_sb")
            nc.sync.dma_start(tri_sb[:], tri[:])
            nc.sync.dma_start(maskA_sb[:], maskA[:])
            nc.sync.dma_start(eye_sb[:], eye[:])
            nc.sync.dma_start(ones_sb[:], onescol[:])

            # CC-stream warmup collective, issued first on gpsimd so the
            # ~50us NRT stream init overlaps the first projection quarter.
            if rs:
                warm_in = dramp.tile([16, 16], BF16, tag="warm_in", name="warm_in")
                warm_out = dramp.tile(
                    [128, 16], BF16, tag="warm_out", name="warm_out",
                    addr_space="Shared",
                )
                warm_src = pp.tile([16, 16], BF16, tag="warm_src", name="warm_src")
                nc.vector.memset(warm_src[:], 0.0)
                nc.gpsimd.dma_start(warm_in[:], warm_src[:])
                nc.gpsimd.collective_compute(
                    "AllGather",
                    mybir.AluOpType.bypass,
                    replica_groups=[list(range(N_CORES))],
                    ins=[warm_in.opt()],
                    outs=[warm_out.opt()],
                )

            # weight prefetch: qkv weights issue on SP in consumption order
            # (interleaved per chunk, pacing the quarter-0 matmuls); wo on
            # gpsimd (needed only from the second slot on).
            wo_t = []
            for c in range(KC):
                tw = sp.tile([128, DQ], BF16, tag=f"woc{c}", name=f"woc{c}")
                nc.gpsimd.dma_start(tw[:], wo[c * 128 : (c + 1) * 128, :])
                wo_t.append(tw)
            wq_t, wk_t, wv_t = [], [], []
            for c in range(KC):
                crow = slice(c * 128, (c + 1) * 128)
                tq = sp.tile([128, DQ], BF16, tag=f"wqc{c}", name=f"wqc{c}")
                tk = sp.tile([128, HD], BF16, tag=f"wkc{c}", name=f"wkc{c}")
                tv = sp.tile([128, HD], BF16, tag=f"wvc{c}", name=f"wvc{c}")
                nc.sync.dma_start(tq[:], wq[crow, :])
                nc.sync.dma_start(tk[:], wk[crow, :])
                nc.sync.dma_start(tv[:], wv[crow, :])
                wq_t.append(tq)
                wk_t.append(tk)
                wv_t.append(tv)

            vT = sp.tile([128, S], F32, tag="vT", name="vT")
            ag_ins = [
                dramp.tile([DQ, 256], BF16, tag=f"agin{c}", name=f"agin{c}")
                for c in range(NCH)
            ]
            ag_outs = [
                dramp.tile(
                    [H, 256], BF16, tag=f"agout{c}", name=f"agout{c}",
                    addr_space="Shared",
                )
                for c in range(NCH)
            ]

            def attn_chunk(i: int, apool):
                q0 = i * 256
                js = _pair_js(i)
                L = len(js)
                for h in range(HQ):
                    e_t = ep.tile([128, L * 256], BF16, tag="e", name="e_t")
                    oT = apool.tile([128, 256], F32, tag="oT", name="oT", bufs=2)
                    sm = apool.tile([1, 256], F32, tag="sm", name="sm", bufs=1)

                    spans = []
                    for j in js:
                        left = (j == 0) or (j <= 2 * i <= j + 7)
                        right = (j == 0) or (j <= 2 * i + 1 <= j + 7)
                        qs = q0 if left else q0 + 128
                        qe = q0 + 256 if right else q0 + 128
                        spans.append((qs, qe))

                    def score(idx: int):
                        j = js[idx]
                        qs, qe = spans[idx]
                        w = qe - qs
                        ecols = slice(idx * 256, idx * 256 + w)
                        s_ps = apool.tile(
                            [128, 256], F32, tag="sps", name="s_ps", bufs=3
                        )
                        kq_, kc_ = j // 4, (j % 4) * 128
                        qq_ = qs // QW
                        nc.tensor.matmul(
                            s_ps[:, 0:w],
                            kTr[kq_][:, kc_ : kc_ + 128],
                            qTr[h][qq_][:, qs - qq_ * QW : qe - qq_ * QW],
                            start=True,
                            stop=True,
                        )
                        nc.scalar.activation(
                            e_t[:, ecols],
                            s_ps[:, 0:w],
                            mybir.ActivationFunctionType.Exp,
                            scale=SCALE,
                        )
                        if j == 2 * i:
                            nc.vector.tensor_mul(
                                e_t[:, ecols], e_t[:, ecols], maskA_sb[:]
                            )
                        elif j == 2 * i + 1:
                            nc.vector.tensor_mul(
                                e_t[:, ecols], e_t[:, ecols], tri_sb[:]
                            )

                    def av(idx: int):
                        j = js[idx]
                        qs, qe = spans[idx]
                        w = qe - qs
                        ecols = slice(idx * 256, idx * 256 + w)
                        st, sp_ = (idx == 0), (idx == L - 1)
                        nc.tensor.matmul(
                            oT[:, qs - q0 : qe - q0],
                            vNat[j // 4][:, (j % 4) * 128 : (j % 4 + 1) * 128],
                            e_t[:, ecols],
                            start=st,
                            stop=sp_,
                        )
                        nc.tensor.matmul(
                            sm[:, qs - q0 : qe - q0],
                            ones_sb[:],
                            e_t[:, ecols],
                            start=st,
                            stop=sp_,
                        )

                    score(0)
                    if L > 1:
                        score(1)
                    for idx in range(L):
                        if idx + 2 < L:
                            score(idx + 2)
                        av(idx)

                    r_sb = asb.tile([1, 256], F32, tag="r", name="r_sb")
                    nc.vector.reciprocal_approx_fast(r_sb[:], sm[:])
                    rb = asb.tile([128, 256], F32, tag="rb", name="rb")
                    nc.gpsimd.partition_broadcast(rb[:], r_sb[:])
                    at_c = asb.tile([128, 256], BF16, tag=f"at{h}", name=f"at{h}")
                    nc.vector.tensor_mul(at_c[:], oT[:], rb[:])
                    nc.sync.dma_start(
                        ag_ins[i][h * 128 : (h + 1) * 128, :], at_c[:]
                    )

                if rs:
                    nc.gpsimd.collective_compute(
                        "AllGather",
                        mybir.AluOpType.bypass,
                        replica_groups=[list(range(N_CORES))],
                        ins=[ag_ins[i].opt()],
                        outs=[ag_outs[i].opt()],
                    )
                else:
                    nc.sync.dma_start(ag_outs[i][0:DQ, :], ag_ins[i][:])

            def oproj_chunk(i: int, oppool):
                q0 = i * 256
                ps01 = [
                    oppool.tile(
                        [128, 512], F32, tag=f"op{sb}", name=f"op{sb}", bufs=1
                    )
                    for sb in range(2)
                ]
                # ag loads issue on SP: with the 2-chunk o_proj lag a full
                # projection quarter separates them from their AllGather, so
                # they never head-of-line-block the queue; keeping them off
                # gpsimd keeps the attention broadcasts (AG critical path)
                # unblocked there.
                for c in range(KC):
                    ag_sb = evp.tile(
                        [128, 256], BF16, tag="ag_sb", name="ag_sb", bufs=6
                    )
                    nc.sync.dma_start(
                        ag_sb[:], ag_outs[i][c * 128 : (c + 1) * 128, :]
                    )
                    for sb in range(2):
                        nc.tensor.matmul(
                            ps01[sb][:],
                            ag_sb[:, sb * 128 : (sb + 1) * 128],
                            wo_t[c][:],
                            start=(c == 0),
                            stop=(c == KC - 1),
                        )
                for sb in range(2):
                    ev = evp.tile([128, 512], F32, tag="ev", name="ev")
                    nc.scalar.copy(ev[:], ps01[sb][:])
                    nc.sync.dma_start(
                        out[q0 + sb * 128 : q0 + (sb + 1) * 128, :], ev[:]
                    )

            for nq in range(NQUART):
                ncols = slice(nq * QW, (nq + 1) * QW)
                with tc.tile_pool(
                    name=f"qps{rep}_{nq}", bufs=1, space="PSUM"
                ) as qpool:
                    ps_q = [
                        qpool.tile([128, QW], F32, tag=f"psq{h}", name=f"psq{h}")
                        for h in range(HQ)
                    ]
                    ps_k = qpool.tile([128, QW], F32, tag="psk", name="ps_k")
                    ps_v = qpool.tile([128, QW], F32, tag="psv", name="ps_v")
                    # quarter 0's hid stream issues on ACT so it doesn't sit
                    # behind the 96 weight DMAs on the SP queue
                    hid_eng = nc.scalar if nq == 0 else nc.sync
                    for c in range(KC):
                        crow = slice(c * 128, (c + 1) * 128)
                        hid_c = stp.tile([128, QW], BF16, tag="hid", name="hid_c")
                        hid_eng.dma_start(hid_c[:], hidT[crow, ncols])
                        st, sp_ = (c == 0), (c == KC - 1)
                        for h in range(HQ):
                            nc.tensor.matmul(
                                ps_q[h][:],
                                wq_t[c][:, h * HD : (h + 1) * HD],
                                hid_c[:],
                                start=st,
                                stop=sp_,
                            )
                        nc.tensor.matmul(
                            ps_k[:], wk_t[c][:], hid_c[:], start=st, stop=sp_
                        )
                        nc.tensor.matmul(
                            ps_v[:], wv_t[c][:], hid_c[:], start=st, stop=sp_
                        )

                    cos_sb = stp.tile([128, QW], F32, tag="cos", name="cos_sb", bufs=2)
                    sin_sb = stp.tile([128, QW], F32, tag="sin", name="sin_sb", bufs=2)
                    nc.sync.dma_start(cos_sb[:], cosF[:, ncols])
                    nc.sync.dma_start(sin_sb[:], sinS[:, ncols])

                    # Evacuate PSUM on ACT and RoPE on DVE, grouped per
                    # tensor with q0 first then k: the first attention chunk
                    # needs qTr[0] and the fresh kTr earliest. swp DMAs issue
                    # on ACT right after the raw copy they read.
                    rope_list = [(ps_q[0], qTr[0][nq], "q0"), (ps_k, kTr[nq], "k")]
                    rope_list += [
                        (ps_q[h], qTr[h][nq], f"q{h}") for h in range(1, HQ)
                    ]
                    for ps_x, dstT, tag in rope_list:
                        raw = sp.tile([128, QW], F32, tag=f"raw{tag}", name=f"raw{tag}")
                        nc.scalar.copy(raw[:], ps_x[:])
                        swp = sp.tile([128, QW], F32, tag=f"swp{tag}", name=f"swp{tag}")
                        nc.scalar.dma_start(swp[0:64, :], raw[64:128, :])
                        nc.scalar.dma_start(swp[64:128, :], raw[0:64, :])
                        t1 = sp.tile([128, QW], F32, tag=f"t1{tag}", name=f"t1{tag}")
                        nc.vector.tensor_mul(t1[:], raw[:], cos_sb[:])
                        nc.vector.tensor_mul(swp[:], swp[:], sin_sb[:])
                        nc.vector.tensor_add(dstT[:], t1[:], swp[:])
                    nc.scalar.copy(vT[:, ncols], ps_v[:])

                    # V natural blocks for this quarter (4 transposes)
                    with tc.tile_pool(
                        name=f"trp{rep}_{nq}", bufs=2, space="PSUM"
                    ) as trpool:
                        for jb in range(nq * QW // 128, (nq + 1) * QW // 128):
                            bcols = slice(jb * 128, (jb + 1) * 128)
                            lcols = slice((jb % 4) * 128, (jb % 4 + 1) * 128)
                            tr = trpool.tile([128, 128], F32, tag="tr", name="tr")
                            nc.tensor.transpose(tr[:], vT[:, bcols], eye_sb[:])
                            nc.scalar.copy(vNat[nq][:, lcols], tr[:])

                # attention chunks unlocked by this quarter; o_proj lags by
                # 2 chunks so each AllGather hides under subsequent PE work.
                # The last slot runs chunk 7 before 6 so AG7 is covered by
                # attn(6) + o_proj work instead of sticking out as a tail.
                # Each slot starts with the lagged o_proj (depends only on an
                # old AllGather), covering this quarter's RoPE latency before
                # attention needs the fresh qTr/kTr.
                ca, cb = (2 * nq, 2 * nq + 1) if nq < 3 else (7, 6)
                with tc.tile_pool(
                    name=f"aps{rep}_{nq}", bufs=1, space="PSUM"
                ) as apool:
                    if 2 * nq - 2 >= 0:
                        with tc.tile_pool(
                            name=f"ops{rep}_{nq}a", bufs=1, space="PSUM"
                        ) as oppool:
                            oproj_chunk(2 * nq - 2, oppool)
                    attn_chunk(ca, apool)
                    if 2 * nq - 1 >= 0:
                        with tc.tile_pool(
                            name=f"ops{rep}_{nq}b", bufs=1, space="PSUM"
                        ) as oppool:
                            oproj_chunk(2 * nq - 1, oppool)
                    attn_chunk(cb, apool)

            with tc.tile_pool(
                name=f"ops{rep}_t7", bufs=1, space="PSUM"
            ) as oppool:
                oproj_chunk(NCH - 1, oppool)
            with tc.tile_pool(
                name=f"ops{rep}_t6", bufs=1, space="PSUM"
            ) as oppool:
                oproj_chunk(NCH - 2, oppool)
    nc.compile()
    return nc


@functools.lru_cache(maxsize=1)
def _cached_nc():
    return build_nc(rs=True)


def _tables():
    pos = np.arange(S, dtype=np.float64)
    inv = 1.0 / (ROPE_BASE ** (np.arange(0, HD, 2, dtype=np.float64) / HD))  # [64]
    f = inv[:, None] * pos[None, :]                   # [64, S]
    cos = np.cos(f).astype(np.float32)
    sin = np.sin(f).astype(np.float32)
    cosF = np.concatenate([cos, cos], axis=0)         # [128, S]
    sinS = np.concatenate([-sin, sin], axis=0)        # [128, S]
    k_idx = np.arange(128)[:, None]
    q_idx = np.arange(128)[None, :]
    tri = (k_idx <= q_idx).astype(np.float32)         # [k, q] causal in-block
    eye = np.eye(128, dtype=np.float32)
    maskA = np.concatenate([tri, np.ones((128, 128), np.float32)], axis=1)
    return cosF, sinS, tri, eye, maskA


def _bf16(x: np.ndarray) -> np.ndarray:
    return np.ascontiguousarray(x).astype(ml_dtypes.bfloat16)


def kernel(hidden_states, wq, wk, wv, wo):
    nc = _cached_nc()
    hidT = _bf16(np.asarray(hidden_states, dtype=np.float32).reshape(S, H).T)
    cosF, sinS, tri, eye, maskA = _tables()
    in_maps = []
    for c in range(N_CORES):
        in_maps.append(
            {
                "hidT": hidT,
                "wq": _bf16(wq[:, c * DQ : (c + 1) * DQ]),
                "wk": _bf16(wk[:, c * HD : (c + 1) * HD]),
                "wv": _bf16(wv[:, c * HD : (c + 1) * HD]),
                "wo": _bf16(wo[:, c * DQ : (c + 1) * DQ]),
                "cosF": cosF,
                "sinS": sinS,
                "tri": _bf16(tri),
                "eye": eye,
                "onescol": np.ones((128, 1), dtype=ml_dtypes.bfloat16),
                "maskA": _bf16(maskA),
            }
        )
    kw = dict(trace=True, **TRACE_KW) if TRACE else {}
    res = run_bass_kernel_spmd(nc, in_maps, core_ids=list(range(N_CORES)), **kw)
    global LAST_RESULTS
    LAST_RESULTS = res
    full = np.concatenate(
        [res.results[r]["out"] for r in range(N_CORES)], axis=1
    )
    return full.reshape(B, S, H)


# revision 17
# speedup vs baseline: 1.0670x; 1.0018x over previous
"""Trainium2 Bass kernel for nn_LlamaAttention_48816598286577.

Llama attention with block-streaming sparse mask (sink=1 block, local
window=8 blocks, BLOCK=128), B=1 S=2048 H=4096, 32 q heads / 8 kv heads,
head_dim 128, non-interleaved RoPE.

Sharding: tensor-parallel over heads across 8 cores (4 q heads + 1 kv
head per core). All matmul operands are bf16 (f32r measured ~2x slower
per row and ~4x slower LDWEIGHTS on HW); accumulation is f32 in PSUM.

Schedule: the S=2048 sequence is processed in 4 projection quarters.
After each quarter's QKV+RoPE, the two 256-query attention chunks it
unlocks run immediately, each followed by its bf16 AllGather; o_proj
for chunk i runs two chunks later, so every AllGather hides under
attention + o_proj + next-quarter PE work. DMA descriptor generation is
spread across the Sync/Scalar/Vector/GpSimd queues to avoid the
in-order SP queue serializing issue (565ns each).
"""

import functools
import numpy as np
import ml_dtypes

import concourse.bass as bass
import concourse.mybir as mybir
import concourse.tile as tile
from concourse import bacc
from concourse.bass_utils import run_bass_kernel_spmd

# problem constants (hardcoded per contract)
B, S, H = 1, 2048, 4096
NQ, NKV, HD = 32, 8, 128
BLOCK = 128
NBLK = S // BLOCK          # 16
SINK_BLOCKS = 1
LOCAL_BLOCKS = 8
ROPE_BASE = 10000.0
N_CORES = 8
HQ = NQ // N_CORES         # 4 q heads per core
DQ = HQ * HD               # 512 q columns per core
SCALE = 1.0 / float(np.sqrt(HD))

KC = H // 128              # 32 contraction chunks for projections
NQUART = 4                 # S split into 4 quarters of 512 for projections
QW = S // NQUART           # 512
NCH = NBLK // 2            # 8 attention chunks of 256 queries

F32 = mybir.dt.float32
BF16 = mybir.dt.bfloat16

# Opt-in profiling plumbing (off by default; harness never touches these).
TRACE = False
TRACE_KW: dict = {}
LAST_RESULTS = None


def _pair_js(i: int) -> list[int]:
    """Key blocks contributing to query pair i (blocks 2i, 2i+1)."""
    return sorted(set([0]) | set(range(max(0, 2 * i - 7), 2 * i + 2)))


def build_nc(rs: bool = True, repeat: int = 1):
    nc = bacc.Bacc(
        "TRN2", target_bir_lowering=False, debug=False, num_devices=N_CORES
    )
    hidT = nc.dram_tensor("hidT", [H, S], BF16, kind="ExternalInput").ap()
    wq = nc.dram_tensor("wq", [H, DQ], BF16, kind="ExternalInput").ap()
    wk = nc.dram_tensor("wk", [H, HD], BF16, kind="ExternalInput").ap()
    wv = nc.dram_tensor("wv", [H, HD], BF16, kind="ExternalInput").ap()
    wo = nc.dram_tensor("wo", [H, DQ], BF16, kind="ExternalInput").ap()
    cosF = nc.dram_tensor("cosF", [128, S], F32, kind="ExternalInput").ap()
    sinS = nc.dram_tensor("sinS", [128, S], F32, kind="ExternalInput").ap()
    tri = nc.dram_tensor("tri", [128, 128], BF16, kind="ExternalInput").ap()
    eye = nc.dram_tensor("eye", [128, 128], F32, kind="ExternalInput").ap()
    onescol = nc.dram_tensor("onescol", [128, 1], BF16, kind="ExternalInput").ap()
    maskA = nc.dram_tensor("maskA", [128, 256], BF16, kind="ExternalInput").ap()
    out = nc.dram_tensor("out", [S, DQ], F32, kind="ExternalOutput").ap()

    with tile.TileContext(nc) as tc:
      for rep in range(repeat):
        with (
            tc.tile_pool(name=f"persist{rep}", bufs=1) as pp,
            tc.tile_pool(name=f"dram{rep}", bufs=1, space="DRAM") as dramp,
            tc.tile_pool(name=f"sp{rep}", bufs=1) as sp,
            tc.tile_pool(name=f"stream{rep}", bufs=3) as stp,
            tc.tile_pool(name=f"e_sb{rep}", bufs=3) as ep,
            tc.tile_pool(name=f"att_sb{rep}", bufs=2) as asb,
            tc.tile_pool(name=f"ev_sb{rep}", bufs=3) as evp,
        ):
            qTr = [
                [
                    pp.tile([128, QW], BF16, tag=f"qTr{h}_{nq}", name=f"qTr{h}_{nq}")
                    for nq in range(NQUART)
                ]
                for h in range(HQ)
            ]
            kTr = [
                pp.tile([128, QW], BF16, tag=f"kTr{nq}", name=f"kTr{nq}")
                for nq in range(NQUART)
            ]
            vNat = [
                pp.tile([128, QW], BF16, tag=f"vNat{nq}", name=f"vNat{nq}")
                for nq in range(NQUART)
            ]
            tri_sb = pp.tile([128, 128], BF16, tag="tri", name="tri_sb")
            maskA_sb = pp.tile([128, 256], BF16, tag="maskA", name="maskA_sb")
            eye_sb = pp.tile([128, 128], F32, tag="eye", name="eye_sb")
            ones_sb = pp.tile([128, 1], BF16, tag="ones", name="ones_sb")
            nc.sync.dma_start(tri_sb[:], tri[:])
            nc.sync.dma_start(maskA_sb[:], maskA[:])
            nc.sync.dma_start(eye_sb[:], eye[:])
            nc.sync.dma_start(ones_sb[:], onescol[:])

            # CC-stream warmup collective, issued first on gpsimd so the
            # ~50us NRT stream init overlaps the first projection quarter.
            if rs:
                warm_in = dramp.tile([16, 16], BF16, tag="warm_in", name="warm_in")
                warm_out = dramp.tile(
                    [128, 16], BF16, tag="warm_out", name="warm_out",
                    addr_space="Shared",
                )
                warm_src = pp.tile([16, 16], BF16, tag="warm_src", name="warm_src")
                nc.vector.memset(warm_src[:], 0.0)
                nc.gpsimd.dma_start(warm_in[:], warm_src[:])
                nc.gpsimd.collective_compute(
                    "AllGather",
                    mybir.AluOpType.bypass,
                    replica_groups=[list(range(N_CORES))],
                    ins=[warm_in.opt()],
                    outs=[warm_out.opt()],
                    unique_tensors="Yes",
                )

            # weight prefetch: qkv weights issue on SP in consumption order
            # (interleaved per chunk, pacing the quarter-0 matmuls); wo on
            # gpsimd (needed only from the second slot on).
            wo_t = []
            for c in range(KC):
                tw = sp.tile([128, DQ], BF16, tag=f"woc{c}", name=f"woc{c}")
                nc.gpsimd.dma_start(tw[:], wo[c * 128 : (c + 1) * 128, :])
                wo_t.append(tw)
            wq_t, wk_t, wv_t = [], [], []
            for c in range(KC):
                crow = slice(c * 128, (c + 1) * 128)
                tq = sp.tile([128, DQ], BF16, tag=f"wqc{c}", name=f"wqc{c}")
                tk = sp.tile([128, HD], BF16, tag=f"wkc{c}", name=f"wkc{c}")
                tv = sp.tile([128, HD], BF16, tag=f"wvc{c}", name=f"wvc{c}")
                nc.sync.dma_start(tq[:], wq[crow, :])
                nc.sync.dma_start(tk[:], wk[crow, :])
                nc.sync.dma_start(tv[:], wv[crow, :])
                wq_t.append(tq)
                wk_t.append(tk)
                wv_t.append(tv)

            vT = sp.tile([128, S], F32, tag="vT", name="vT")
            ag_ins = [
                dramp.tile([DQ, 256], BF16, tag=f"agin{c}", name=f"agin{c}")
                for c in range(NCH)
            ]
            ag_outs = [
                dramp.tile(
                    [H, 256], BF16, tag=f"agout{c}", name=f"agout{c}",
                    addr_space="Shared",
                )
                for c in range(NCH)
            ]

            def attn_chunk(i: int, apool):
                q0 = i * 256
                js = _pair_js(i)
                L = len(js)
                for h in range(HQ):
                    e_t = ep.tile([128, L * 256], BF16, tag="e", name="e_t")
                    oT = apool.tile([128, 256], F32, tag="oT", name="oT", bufs=2)
                    sm = apool.tile([1, 256], F32, tag="sm", name="sm", bufs=1)

                    spans = []
                    for j in js:
                        left = (j == 0) or (j <= 2 * i <= j + 7)
                        right = (j == 0) or (j <= 2 * i + 1 <= j + 7)
                        qs = q0 if left else q0 + 128
                        qe = q0 + 256 if right else q0 + 128
                        spans.append((qs, qe))

                    def score(idx: int):
                        j = js[idx]
                        qs, qe = spans[idx]
                        w = qe - qs
                        ecols = slice(idx * 256, idx * 256 + w)
                        s_ps = apool.tile(
                            [128, 256], F32, tag="sps", name="s_ps", bufs=3
                        )
                        kq_, kc_ = j // 4, (j % 4) * 128
                        qq_ = qs // QW
                        nc.tensor.matmul(
                            s_ps[:, 0:w],
                            kTr[kq_][:, kc_ : kc_ + 128],
                            qTr[h][qq_][:, qs - qq_ * QW : qe - qq_ * QW],
                            start=True,
                            stop=True,
                        )
                        nc.scalar.activation(
                            e_t[:, ecols],
                            s_ps[:, 0:w],
                            mybir.ActivationFunctionType.Exp,
                            scale=SCALE,
                        )
                        if j == 2 * i:
                            nc.vector.tensor_mul(
                                e_t[:, ecols], e_t[:, ecols], maskA_sb[:]
                            )
                        elif j == 2 * i + 1:
                            nc.vector.tensor_mul(
                                e_t[:, ecols], e_t[:, ecols], tri_sb[:]
                            )

                    def av(idx: int):
                        j = js[idx]
                        qs, qe = spans[idx]
                        w = qe - qs
                        ecols = slice(idx * 256, idx * 256 + w)
                        st, sp_ = (idx == 0), (idx == L - 1)
                        nc.tensor.matmul(
                            oT[:, qs - q0 : qe - q0],
                            vNat[j // 4][:, (j % 4) * 128 : (j % 4 + 1) * 128],
                            e_t[:, ecols],
                            start=st,
                            stop=sp_,
                        )
                        nc.tensor.matmul(
                            sm[:, qs - q0 : qe - q0],
                            ones_sb[:],
                            e_t[:, ecols],
                            start=st,
                            stop=sp_,
                        )

                    score(0)
                    if L > 1:
                        score(1)
                    for idx in range(L):
                        if idx + 2 < L:
                            score(idx + 2)
                        av(idx)

                    r_sb = asb.tile([1, 256], F32, tag="r", name="r_sb")
                    nc.vector.reciprocal_approx_fast(r_sb[:], sm[:])
                    rb = asb.tile([128, 256], F32, tag="rb", name="rb")
                    nc.gpsimd.partition_broadcast(rb[:], r_sb[:])
                    at_c = asb.tile([128, 256], BF16, tag=f"at{h}", name=f"at{h}")
                    nc.vector.tensor_mul(at_c[:], oT[:], rb[:])
                    nc.sync.dma_start(
                        ag_ins[i][h * 128 : (h + 1) * 128, :], at_c[:]
                    )

                if rs:
                    nc.gpsimd.collective_compute(
                        "AllGather",
                        mybir.AluOpType.bypass,
                        replica_groups=[list(range(N_CORES))],
                        ins=[ag_ins[i].opt()],
                        outs=[ag_outs[i].opt()],
                        unique_tensors="Yes",
                    )
                else:
                    nc.sync.dma_start(ag_outs[i][0:DQ, :], ag_ins[i][:])

            def oproj_chunk(i: int, oppool):
                q0 = i * 256
                ps01 = [
                    oppool.tile(
                        [128, 512], F32, tag=f"op{sb}", name=f"op{sb}", bufs=1
                    )
                    for sb in range(2)
                ]
                # ag loads issue on SP: with the 2-chunk o_proj lag a full
                # projection quarter separates them from their AllGather, so
                # they never head-of-line-block the queue; keeping them off
                # gpsimd keeps the attention broadcasts (AG critical path)
                # unblocked there.
                for c in range(KC):
                    ag_sb = evp.tile(
                        [128, 256], BF16, tag="ag_sb", name="ag_sb", bufs=6
                    )
                    nc.sync.dma_start(
                        ag_sb[:], ag_outs[i][c * 128 : (c + 1) * 128, :]
                    )
                    for sb in range(2):
                        nc.tensor.matmul(
                            ps01[sb][:],
                            ag_sb[:, sb * 128 : (sb + 1) * 128],
                            wo_t[c][:],
                            start=(c == 0),
                            stop=(c == KC - 1),
                        )
                for sb in range(2):
                    ev = evp.tile([128, 512], F32, tag="ev", name="ev")
                    nc.scalar.copy(ev[:], ps01[sb][:])
                    nc.sync.dma_start(
                        out[q0 + sb * 128 : q0 + (sb + 1) * 128, :], ev[:]
                    )

            for nq in range(NQUART):
                ncols = slice(nq * QW, (nq + 1) * QW)
                with tc.tile_pool(
                    name=f"qps{rep}_{nq}", bufs=1, space="PSUM"
                ) as qpool:
                    ps_q = [
                        qpool.tile([128, QW], F32, tag=f"psq{h}", name=f"psq{h}")
                        for h in range(HQ)
                    ]
                    ps_k = qpool.tile([128, QW], F32, tag="psk", name="ps_k")
                    ps_v = qpool.tile([128, QW], F32, tag="psv", name="ps_v")
                    # quarter 0's hid stream issues on ACT so it doesn't sit
                    # behind the 96 weight DMAs on the SP queue
                    hid_eng = nc.scalar if nq == 0 else nc.sync
                    for c in range(KC):
                        crow = slice(c * 128, (c + 1) * 128)
                        hid_c = stp.tile([128, QW], BF16, tag="hid", name="hid_c")
                        hid_eng.dma_start(hid_c[:], hidT[crow, ncols])
                        st, sp_ = (c == 0), (c == KC - 1)
                        for h in range(HQ):
                            nc.tensor.matmul(
                                ps_q[h][:],
                                wq_t[c][:, h * HD : (h + 1) * HD],
                                hid_c[:],
                                start=st,
                                stop=sp_,
                            )
                        nc.tensor.matmul(
                            ps_k[:], wk_t[c][:], hid_c[:], start=st, stop=sp_
                        )
                        nc.tensor.matmul(
                            ps_v[:], wv_t[c][:], hid_c[:], start=st, stop=sp_
                        )

                    cos_sb = stp.tile([128, QW], F32, tag="cos", name="cos_sb", bufs=2)
                    sin_sb = stp.tile([128, QW], F32, tag="sin", name="sin_sb", bufs=2)
                    nc.sync.dma_start(cos_sb[:], cosF[:, ncols])
                    nc.sync.dma_start(sin_sb[:], sinS[:, ncols])

                    # Evacuate PSUM on ACT and RoPE on DVE, grouped per
                    # tensor with q0 first then k: the first attention chunk
                    # needs qTr[0] and the fresh kTr earliest. swp DMAs issue
                    # on ACT right after the raw copy they read.
                    rope_list = [(ps_q[0], qTr[0][nq], "q0"), (ps_k, kTr[nq], "k")]
                    rope_list += [
                        (ps_q[h], qTr[h][nq], f"q{h}") for h in range(1, HQ)
                    ]
                    for ps_x, dstT, tag in rope_list:
                        raw = sp.tile([128, QW], F32, tag=f"raw{tag}", name=f"raw{tag}")
                        nc.scalar.copy(raw[:], ps_x[:])
                        swp = sp.tile([128, QW], F32, tag=f"swp{tag}", name=f"swp{tag}")
                        nc.scalar.dma_start(swp[0:64, :], raw[64:128, :])
                        nc.scalar.dma_start(swp[64:128, :], raw[0:64, :])
                        t1 = sp.tile([128, QW], F32, tag=f"t1{tag}", name=f"t1{tag}")
                        nc.vector.tensor_mul(t1[:], raw[:], cos_sb[:])
                        nc.vector.tensor_mul(swp[:], swp[:], sin_sb[:])
                        nc.vector.tensor_add(dstT[:], t1[:], swp[:])
                    nc.scalar.copy(vT[:, ncols], ps_v[:])

                    # V natural blocks for this quarter (4 transposes)
                    with tc.tile_pool(
                        name=f"trp{rep}_{nq}", bufs=2, space="PSUM"
                    ) as trpool:
                        for jb in range(nq * QW // 128, (nq + 1) * QW // 128):
                            bcols = slice(jb * 128, (jb + 1) * 128)
                            lcols = slice((jb % 4) * 128, (jb % 4 + 1) * 128)
                            tr = trpool.tile([128, 128], F32, tag="tr", name="tr")
                            nc.tensor.transpose(tr[:], vT[:, bcols], eye_sb[:])
                            nc.scalar.copy(vNat[nq][:, lcols], tr[:])

                # attention chunks unlocked by this quarter; o_proj lags by
                # 2 chunks so each AllGather hides under subsequent PE work.
                # The last slot runs chunk 7 before 6 so AG7 is covered by
                # attn(6) + o_proj work instead of sticking out as a tail.
                # Each slot starts with the lagged o_proj (depends only on an
                # old AllGather), covering this quarter's RoPE latency before
                # attention needs the fresh qTr/kTr.
                ca, cb = (2 * nq, 2 * nq + 1) if nq < 3 else (7, 6)
                with tc.tile_pool(
                    name=f"aps{rep}_{nq}", bufs=1, space="PSUM"
                ) as apool:
                    if 2 * nq - 2 >= 0:
                        with tc.tile_pool(
                            name=f"ops{rep}_{nq}a", bufs=1, space="PSUM"
                        ) as oppool:
                            oproj_chunk(2 * nq - 2, oppool)
                    attn_chunk(ca, apool)
                    if 2 * nq - 1 >= 0:
                        with tc.tile_pool(
                            name=f"ops{rep}_{nq}b", bufs=1, space="PSUM"
                        ) as oppool:
                            oproj_chunk(2 * nq - 1, oppool)
                    attn_chunk(cb, apool)

            with tc.tile_pool(
                name=f"ops{rep}_t7", bufs=1, space="PSUM"
            ) as oppool:
                oproj_chunk(NCH - 1, oppool)
            with tc.tile_pool(
                name=f"ops{rep}_t6", bufs=1, space="PSUM"
            ) as oppool:
                oproj_chunk(NCH - 2, oppool)
    nc.compile()
    return nc


@functools.lru_cache(maxsize=1)
def _cached_nc():
    return build_nc(rs=True)


def _tables():
    pos = np.arange(S, dtype=np.float64)
    inv = 1.0 / (ROPE_BASE ** (np.arange(0, HD, 2, dtype=np.float64) / HD))  # [64]
    f = inv[:, None] * pos[None, :]                   # [64, S]
    cos = np.cos(f).astype(np.float32)
    sin = np.sin(f).astype(np.float32)
    cosF = np.concatenate([cos, cos], axis=0)         # [128, S]
    sinS = np.concatenate([-sin, sin], axis=0)        # [128, S]
    k_idx = np.arange(128)[:, None]
    q_idx = np.arange(128)[None, :]
    tri = (k_idx <= q_idx).astype(np.float32)         # [k, q] causal in-block
    eye = np.eye(128, dtype=np.float32)
    maskA = np.concatenate([tri, np.ones((128, 128), np.float32)], axis=1)
    return cosF, sinS, tri, eye, maskA


def _bf16(x: np.ndarray) -> np.ndarray:
    return np.ascontiguousarray(x).astype(ml_dtypes.bfloat16)


def kernel(hidden_states, wq, wk, wv, wo):
    nc = _cached_nc()
    hidT = _bf16(np.asarray(hidden_states, dtype=np.float32).reshape(S, H).T)
    cosF, sinS, tri, eye, maskA = _tables()
    in_maps = []
    for c in range(N_CORES):
        in_maps.append(
            {
                "hidT": hidT,
                "wq": _bf16(wq[:, c * DQ : (c + 1) * DQ]),
                "wk": _bf16(wk[:, c * HD : (c + 1) * HD]),
                "wv": _bf16(wv[:, c * HD : (c + 1) * HD]),
                "wo": _bf16(wo[:, c * DQ : (c + 1) * DQ]),
                "cosF": cosF,
                "sinS": sinS,
                "tri": _bf16(tri),
                "eye": eye,
                "onescol": np.ones((128, 1), dtype=ml_dtypes.bfloat16),
                "maskA": _bf16(maskA),
            }
        )
    kw = dict(trace=True, **TRACE_KW) if TRACE else {}
    res = run_bass_kernel_spmd(nc, in_maps, core_ids=list(range(N_CORES)), **kw)
    global LAST_RESULTS
    LAST_RESULTS = res
    full = np.concatenate(
        [res.results[r]["out"] for r in range(N_CORES)], axis=1
    )
    return full.reshape(B, S, H)


# revision 18
# speedup vs baseline: 1.0898x; 1.0213x over previous
"""Trainium2 Bass kernel for nn_LlamaAttention_48816598286577.

Llama attention with block-streaming sparse mask (sink=1 block, local
window=8 blocks, BLOCK=128), B=1 S=2048 H=4096, 32 q heads / 8 kv heads,
head_dim 128, non-interleaved RoPE.

Sharding: tensor-parallel over heads across 8 cores (4 q heads + 1 kv
head per core). All matmul operands are bf16 (f32r measured ~2x slower
per row and ~4x slower LDWEIGHTS on HW); accumulation is f32 in PSUM.

Schedule: the S=2048 sequence is processed in 4 projection quarters.
After each quarter's QKV+RoPE, the two 256-query attention chunks it
unlocks run immediately, each followed by its bf16 AllGather; o_proj
for chunk i runs two chunks later, so every AllGather hides under
attention + o_proj + next-quarter PE work. DMA descriptor generation is
spread across the Sync/Scalar/Vector/GpSimd queues to avoid the
in-order SP queue serializing issue (565ns each).
"""

import functools
import numpy as np
import ml_dtypes

import concourse.bass as bass
import concourse.mybir as mybir
import concourse.tile as tile
from concourse import bacc
from concourse.bass_utils import run_bass_kernel_spmd

# problem constants (hardcoded per contract)
B, S, H = 1, 2048, 4096
NQ, NKV, HD = 32, 8, 128
BLOCK = 128
NBLK = S // BLOCK          # 16
SINK_BLOCKS = 1
LOCAL_BLOCKS = 8
ROPE_BASE = 10000.0
N_CORES = 8
HQ = NQ // N_CORES         # 4 q heads per core
DQ = HQ * HD               # 512 q columns per core
SCALE = 1.0 / float(np.sqrt(HD))

KC = H // 128              # 32 contraction chunks for projections
NQUART = 4                 # S split into 4 quarters of 512 for projections
QW = S // NQUART           # 512
NCH = NBLK // 2            # 8 attention chunks of 256 queries

F32 = mybir.dt.float32
BF16 = mybir.dt.bfloat16

# Opt-in profiling plumbing (off by default; harness never touches these).
TRACE = False
TRACE_KW: dict = {}
LAST_RESULTS = None


def _pair_js(i: int) -> list[int]:
    """Key blocks contributing to query pair i (blocks 2i, 2i+1)."""
    return sorted(set([0]) | set(range(max(0, 2 * i - 7), 2 * i + 2)))


def build_nc(rs: bool = True, repeat: int = 1):
    nc = bacc.Bacc(
        "TRN2", target_bir_lowering=False, debug=False, num_devices=N_CORES
    )
    hidT = nc.dram_tensor("hidT", [H, S], BF16, kind="ExternalInput").ap()
    wq = nc.dram_tensor("wq", [H, DQ], BF16, kind="ExternalInput").ap()
    wk = nc.dram_tensor("wk", [H, HD], BF16, kind="ExternalInput").ap()
    wv = nc.dram_tensor("wv", [H, HD], BF16, kind="ExternalInput").ap()
    wo = nc.dram_tensor("wo", [H, DQ], BF16, kind="ExternalInput").ap()
    cosF = nc.dram_tensor("cosF", [128, S], F32, kind="ExternalInput").ap()
    sinS = nc.dram_tensor("sinS", [128, S], F32, kind="ExternalInput").ap()
    tri = nc.dram_tensor("tri", [128, 128], BF16, kind="ExternalInput").ap()
    eye = nc.dram_tensor("eye", [128, 128], F32, kind="ExternalInput").ap()
    onescol = nc.dram_tensor("onescol", [128, 1], BF16, kind="ExternalInput").ap()
    maskA = nc.dram_tensor("maskA", [128, 256], BF16, kind="ExternalInput").ap()
    out = nc.dram_tensor("out", [S, DQ], F32, kind="ExternalOutput").ap()

    with tile.TileContext(nc) as tc:
      for rep in range(repeat):
        with (
            tc.tile_pool(name=f"persist{rep}", bufs=1) as pp,
            tc.tile_pool(name=f"dram{rep}", bufs=1, space="DRAM") as dramp,
            tc.tile_pool(name=f"sp{rep}", bufs=1) as sp,
            tc.tile_pool(name=f"stream{rep}", bufs=3) as stp,
            tc.tile_pool(name=f"e_sb{rep}", bufs=3) as ep,
            tc.tile_pool(name=f"att_sb{rep}", bufs=2) as asb,
            tc.tile_pool(name=f"ev_sb{rep}", bufs=3) as evp,
        ):
            qTr = [
                [
                    pp.tile([128, QW], BF16, tag=f"qTr{h}_{nq}", name=f"qTr{h}_{nq}")
                    for nq in range(NQUART)
                ]
                for h in range(HQ)
            ]
            kTr = [
                pp.tile([128, QW], BF16, tag=f"kTr{nq}", name=f"kTr{nq}")
                for nq in range(NQUART)
            ]
            vNat = [
                pp.tile([128, QW], BF16, tag=f"vNat{nq}", name=f"vNat{nq}")
                for nq in range(NQUART)
            ]
            tri_sb = pp.tile([128, 128], BF16, tag="tri", name="tri_sb")
            maskA_sb = pp.tile([128, 256], BF16, tag="maskA", name="maskA_sb")
            eye_sb = pp.tile([128, 128], F32, tag="eye", name="eye_sb")
            ones_sb = pp.tile([128, 1], BF16, tag="ones", name="ones_sb")
            nc.sync.dma_start(tri_sb[:], tri[:])
            nc.sync.dma_start(maskA_sb[:], maskA[:])
            nc.sync.dma_start(eye_sb[:], eye[:])
            nc.sync.dma_start(ones_sb[:], onescol[:])

            # CC-stream warmup collective, issued first on gpsimd so the
            # ~50us NRT stream init overlaps the first projection quarter.
            if rs:
                warm_in = dramp.tile([16, 16], BF16, tag="warm_in", name="warm_in")
                warm_out = dramp.tile(
                    [128, 16], BF16, tag="warm_out", name="warm_out",
                    addr_space="Shared",
                )
                warm_src = pp.tile([16, 16], BF16, tag="warm_src", name="warm_src")
                nc.vector.memset(warm_src[:], 0.0)
                nc.gpsimd.dma_start(warm_in[:], warm_src[:])
                nc.gpsimd.collective_compute(
                    "AllGather",
                    mybir.AluOpType.bypass,
                    replica_groups=[list(range(N_CORES))],
                    ins=[warm_in.opt()],
                    outs=[warm_out.opt()],
                    unique_tensors="Yes",
                )

            # weight prefetch: qkv weights issue on SP in consumption order
            # (interleaved per chunk, pacing the quarter-0 matmuls); wo on
            # gpsimd (needed only from the second slot on).
            wo_t = []
            for c in range(KC):
                tw = sp.tile([128, DQ], BF16, tag=f"woc{c}", name=f"woc{c}")
                nc.gpsimd.dma_start(tw[:], wo[c * 128 : (c + 1) * 128, :])
                wo_t.append(tw)
            wq_t, wk_t, wv_t = [], [], []
            for c in range(KC):
                crow = slice(c * 128, (c + 1) * 128)
                tq = sp.tile([128, DQ], BF16, tag=f"wqc{c}", name=f"wqc{c}")
                tk = sp.tile([128, HD], BF16, tag=f"wkc{c}", name=f"wkc{c}")
                tv = sp.tile([128, HD], BF16, tag=f"wvc{c}", name=f"wvc{c}")
                nc.sync.dma_start(tq[:], wq[crow, :])
                nc.sync.dma_start(tk[:], wk[crow, :])
                nc.sync.dma_start(tv[:], wv[crow, :])
                wq_t.append(tq)
                wk_t.append(tk)
                wv_t.append(tv)

            vT = sp.tile([128, S], F32, tag="vT", name="vT")
            ag_ins = [
                dramp.tile([DQ, 256], BF16, tag=f"agin{c}", name=f"agin{c}")
                for c in range(NCH)
            ]
            ag_outs = [
                dramp.tile(
                    [H, 256], BF16, tag=f"agout{c}", name=f"agout{c}",
                    addr_space="Shared",
                )
                for c in range(NCH)
            ]

            def attn_chunk(i: int, apool):
                q0 = i * 256
                js = _pair_js(i)
                L = len(js)
                for h in range(HQ):
                    e_t = ep.tile([128, L * 256], BF16, tag="e", name="e_t")
                    oT = apool.tile([128, 256], F32, tag="oT", name="oT", bufs=2)
                    sm = apool.tile([1, 256], F32, tag="sm", name="sm", bufs=1)

                    spans = []
                    for j in js:
                        left = (j == 0) or (j <= 2 * i <= j + 7)
                        right = (j == 0) or (j <= 2 * i + 1 <= j + 7)
                        qs = q0 if left else q0 + 128
                        qe = q0 + 256 if right else q0 + 128
                        spans.append((qs, qe))

                    def score(idx: int):
                        j = js[idx]
                        qs, qe = spans[idx]
                        w = qe - qs
                        ecols = slice(idx * 256, idx * 256 + w)
                        s_ps = apool.tile(
                            [128, 256], F32, tag="sps", name="s_ps", bufs=3
                        )
                        kq_, kc_ = j // 4, (j % 4) * 128
                        qq_ = qs // QW
                        nc.tensor.matmul(
                            s_ps[:, 0:w],
                            kTr[kq_][:, kc_ : kc_ + 128],
                            qTr[h][qq_][:, qs - qq_ * QW : qe - qq_ * QW],
                            start=True,
                            stop=True,
                        )
                        nc.scalar.activation(
                            e_t[:, ecols],
                            s_ps[:, 0:w],
                            mybir.ActivationFunctionType.Exp,
                            scale=SCALE,
                        )
                        if j == 2 * i:
                            nc.vector.tensor_mul(
                                e_t[:, ecols], e_t[:, ecols], maskA_sb[:]
                            )
                        elif j == 2 * i + 1:
                            nc.vector.tensor_mul(
                                e_t[:, ecols], e_t[:, ecols], tri_sb[:]
                            )

                    def av(idx: int):
                        j = js[idx]
                        qs, qe = spans[idx]
                        w = qe - qs
                        ecols = slice(idx * 256, idx * 256 + w)
                        st, sp_ = (idx == 0), (idx == L - 1)
                        nc.tensor.matmul(
                            oT[:, qs - q0 : qe - q0],
                            vNat[j // 4][:, (j % 4) * 128 : (j % 4 + 1) * 128],
                            e_t[:, ecols],
                            start=st,
                            stop=sp_,
                        )
                        nc.tensor.matmul(
                            sm[:, qs - q0 : qe - q0],
                            ones_sb[:],
                            e_t[:, ecols],
                            start=st,
                            stop=sp_,
                        )

                    score(0)
                    if L > 1:
                        score(1)
                    for idx in range(L):
                        if idx + 2 < L:
                            score(idx + 2)
                        av(idx)

                    r_sb = asb.tile([1, 256], F32, tag="r", name="r_sb")
                    nc.vector.reciprocal_approx_fast(r_sb[:], sm[:])
                    rb = asb.tile([128, 256], F32, tag="rb", name="rb")
                    nc.gpsimd.partition_broadcast(rb[:], r_sb[:])
                    at_c = asb.tile([128, 256], BF16, tag=f"at{h}", name=f"at{h}")
                    nc.vector.tensor_mul(at_c[:], oT[:], rb[:])
                    nc.sync.dma_start(
                        ag_ins[i][h * 128 : (h + 1) * 128, :], at_c[:]
                    )

                if rs:
                    nc.gpsimd.collective_compute(
                        "AllGather",
                        mybir.AluOpType.bypass,
                        replica_groups=[list(range(N_CORES))],
                        ins=[ag_ins[i].opt()],
                        outs=[ag_outs[i].opt()],
                        unique_tensors="Yes",
                    )
                else:
                    nc.sync.dma_start(ag_outs[i][0:DQ, :], ag_ins[i][:])

            def oproj_chunk(i: int, oppool):
                q0 = i * 256
                ps01 = [
                    oppool.tile(
                        [128, 512], F32, tag=f"op{sb}", name=f"op{sb}", bufs=1
                    )
                    for sb in range(2)
                ]
                # ag loads issue on SP: with the 2-chunk o_proj lag a full
                # projection quarter separates them from their AllGather, so
                # they never head-of-line-block the queue; keeping them off
                # gpsimd keeps the attention broadcasts (AG critical path)
                # unblocked there.
                for c in range(KC):
                    ag_sb = evp.tile(
                        [128, 256], BF16, tag="ag_sb", name="ag_sb", bufs=6
                    )
                    nc.sync.dma_start(
                        ag_sb[:], ag_outs[i][c * 128 : (c + 1) * 128, :]
                    )
                    for sb in range(2):
                        nc.tensor.matmul(
                            ps01[sb][:],
                            ag_sb[:, sb * 128 : (sb + 1) * 128],
                            wo_t[c][:],
                            start=(c == 0),
                            stop=(c == KC - 1),
                        )
                for sb in range(2):
                    ev = evp.tile([128, 512], F32, tag="ev", name="ev")
                    nc.scalar.copy(ev[:], ps01[sb][:])
                    nc.sync.dma_start(
                        out[q0 + sb * 128 : q0 + (sb + 1) * 128, :], ev[:]
                    )

            for nq in range(NQUART):
                ncols = slice(nq * QW, (nq + 1) * QW)
                with tc.tile_pool(
                    name=f"qps{rep}_{nq}", bufs=1, space="PSUM"
                ) as qpool:
                    ps_q = [
                        qpool.tile([128, QW], F32, tag=f"psq{h}", name=f"psq{h}")
                        for h in range(HQ)
                    ]
                    ps_k = qpool.tile([128, QW], F32, tag="psk", name="ps_k")
                    ps_v = qpool.tile([128, QW], F32, tag="psv", name="ps_v")
                    # quarter 0's hid stream issues on ACT so it doesn't sit
                    # behind the 96 weight DMAs on the SP queue
                    hid_eng = nc.scalar if nq == 0 else nc.sync
                    for c in range(KC):
                        crow = slice(c * 128, (c + 1) * 128)
                        hid_c = stp.tile([128, QW], BF16, tag="hid", name="hid_c")
                        hid_eng.dma_start(hid_c[:], hidT[crow, ncols])
                        st, sp_ = (c == 0), (c == KC - 1)
                        for h in range(HQ):
                            nc.tensor.matmul(
                                ps_q[h][:],
                                wq_t[c][:, h * HD : (h + 1) * HD],
                                hid_c[:],
                                start=st,
                                stop=sp_,
                            )
                        nc.tensor.matmul(
                            ps_k[:], wk_t[c][:], hid_c[:], start=st, stop=sp_
                        )
                        nc.tensor.matmul(
                            ps_v[:], wv_t[c][:], hid_c[:], start=st, stop=sp_
                        )

                    cos_sb = stp.tile([128, QW], F32, tag="cos", name="cos_sb", bufs=2)
                    sin_sb = stp.tile([128, QW], F32, tag="sin", name="sin_sb", bufs=2)
                    nc.sync.dma_start(cos_sb[:], cosF[:, ncols])
                    nc.sync.dma_start(sin_sb[:], sinS[:, ncols])

                    # Evacuate PSUM on ACT and RoPE on DVE, grouped per
                    # tensor with q0 first then k: the first attention chunk
                    # needs qTr[0] and the fresh kTr earliest. swp DMAs issue
                    # on ACT right after the raw copy they read.
                    rope_list = [(ps_q[0], qTr[0][nq], "q0"), (ps_k, kTr[nq], "k")]
                    rope_list += [
                        (ps_q[h], qTr[h][nq], f"q{h}") for h in range(1, HQ)
                    ]
                    for ps_x, dstT, tag in rope_list:
                        raw = sp.tile([128, QW], F32, tag=f"raw{tag}", name=f"raw{tag}")
                        nc.scalar.copy(raw[:], ps_x[:])
                        swp = sp.tile([128, QW], F32, tag=f"swp{tag}", name=f"swp{tag}")
                        nc.scalar.dma_start(swp[0:64, :], raw[64:128, :])
                        nc.scalar.dma_start(swp[64:128, :], raw[0:64, :])
                        t1 = sp.tile([128, QW], F32, tag=f"t1{tag}", name=f"t1{tag}")
                        nc.vector.tensor_mul(t1[:], raw[:], cos_sb[:])
                        nc.vector.tensor_mul(swp[:], swp[:], sin_sb[:])
                        nc.vector.tensor_add(dstT[:], t1[:], swp[:])
                    nc.scalar.copy(vT[:, ncols], ps_v[:])

                    # V natural blocks for this quarter (4 transposes)
                    with tc.tile_pool(
                        name=f"trp{rep}_{nq}", bufs=2, space="PSUM"
                    ) as trpool:
                        for jb in range(nq * QW // 128, (nq + 1) * QW // 128):
                            bcols = slice(jb * 128, (jb + 1) * 128)
                            lcols = slice((jb % 4) * 128, (jb % 4 + 1) * 128)
                            tr = trpool.tile([128, 128], F32, tag="tr", name="tr")
                            nc.tensor.transpose(tr[:], vT[:, bcols], eye_sb[:])
                            nc.scalar.copy(vNat[nq][:, lcols], tr[:])

                # attention chunks unlocked by this quarter; o_proj lags by
                # 2 chunks so each AllGather hides under subsequent PE work.
                # The last slot runs chunk 7 before 6 so AG7 is covered by
                # attn(6) + o_proj work instead of sticking out as a tail.
                # attention chunks unlocked by this quarter; o_proj lags by
                # 2 chunks so each AllGather hides under subsequent PE work.
                # The last slot runs chunk 7 before 6 so AG7 is covered by
                # attn(6) + o_proj work instead of sticking out as a tail.
                ca, cb = (2 * nq, 2 * nq + 1) if nq < 3 else (7, 6)
                with tc.tile_pool(
                    name=f"aps{rep}_{nq}", bufs=1, space="PSUM"
                ) as apool:
                    attn_chunk(ca, apool)
                    if 2 * nq - 2 >= 0:
                        with tc.tile_pool(
                            name=f"ops{rep}_{nq}a", bufs=1, space="PSUM"
                        ) as oppool:
                            oproj_chunk(2 * nq - 2, oppool)
                    attn_chunk(cb, apool)
                    if 2 * nq - 1 >= 0:
                        with tc.tile_pool(
                            name=f"ops{rep}_{nq}b", bufs=1, space="PSUM"
                        ) as oppool:
                            oproj_chunk(2 * nq - 1, oppool)

            with tc.tile_pool(
                name=f"ops{rep}_t7", bufs=1, space="PSUM"
            ) as oppool:
                oproj_chunk(NCH - 1, oppool)
            with tc.tile_pool(
                name=f"ops{rep}_t6", bufs=1, space="PSUM"
            ) as oppool:
                oproj_chunk(NCH - 2, oppool)
    nc.compile()
    return nc


@functools.lru_cache(maxsize=1)
def _cached_nc():
    return build_nc(rs=True)


def _tables():
    pos = np.arange(S, dtype=np.float64)
    inv = 1.0 / (ROPE_BASE ** (np.arange(0, HD, 2, dtype=np.float64) / HD))  # [64]
    f = inv[:, None] * pos[None, :]                   # [64, S]
    cos = np.cos(f).astype(np.float32)
    sin = np.sin(f).astype(np.float32)
    cosF = np.concatenate([cos, cos], axis=0)         # [128, S]
    sinS = np.concatenate([-sin, sin], axis=0)        # [128, S]
    k_idx = np.arange(128)[:, None]
    q_idx = np.arange(128)[None, :]
    tri = (k_idx <= q_idx).astype(np.float32)         # [k, q] causal in-block
    eye = np.eye(128, dtype=np.float32)
    maskA = np.concatenate([tri, np.ones((128, 128), np.float32)], axis=1)
    return cosF, sinS, tri, eye, maskA


def _bf16(x: np.ndarray) -> np.ndarray:
    return np.ascontiguousarray(x).astype(ml_dtypes.bfloat16)


def kernel(hidden_states, wq, wk, wv, wo):
    nc = _cached_nc()
    hidT = _bf16(np.asarray(hidden_states, dtype=np.float32).reshape(S, H).T)
    cosF, sinS, tri, eye, maskA = _tables()
    in_maps = []
    for c in range(N_CORES):
        in_maps.append(
            {
                "hidT": hidT,
                "wq": _bf16(wq[:, c * DQ : (c + 1) * DQ]),
                "wk": _bf16(wk[:, c * HD : (c + 1) * HD]),
                "wv": _bf16(wv[:, c * HD : (c + 1) * HD]),
                "wo": _bf16(wo[:, c * DQ : (c + 1) * DQ]),
                "cosF": cosF,
                "sinS": sinS,
                "tri": _bf16(tri),
                "eye": eye,
                "onescol": np.ones((128, 1), dtype=ml_dtypes.bfloat16),
                "maskA": _bf16(maskA),
            }
        )
    kw = dict(trace=True, **TRACE_KW) if TRACE else {}
    res = run_bass_kernel_spmd(nc, in_maps, core_ids=list(range(N_CORES)), **kw)
    global LAST_RESULTS
    LAST_RESULTS = res
    full = np.concatenate(
        [res.results[r]["out"] for r in range(N_CORES)], axis=1
    )
    return full.reshape(B, S, H)


# revision 27
# speedup vs baseline: 1.1276x; 1.0347x over previous
"""Trainium2 Bass kernel for nn_LlamaAttention_48816598286577.

Llama attention with block-streaming sparse mask (sink=1 block, local
window=8 blocks, BLOCK=128), B=1 S=2048 H=4096, 32 q heads / 8 kv heads,
head_dim 128, non-interleaved RoPE.

Sharding: tensor-parallel over heads across 8 cores (4 q heads + 1 kv
head per core). All matmul operands are bf16 (f32r measured ~2x slower
per row and ~4x slower LDWEIGHTS on HW); accumulation is f32 in PSUM.

Schedule: the S=2048 sequence is processed in 4 projection quarters.
After each quarter's QKV+RoPE, the two 256-query attention chunks it
unlocks run immediately, each followed by its bf16 AllGather; o_proj
for chunk i runs two chunks later, so every AllGather hides under
attention + o_proj + next-quarter PE work. DMA descriptor generation is
spread across the Sync/Scalar/Vector/GpSimd queues to avoid the
in-order SP queue serializing issue (565ns each).
"""

import functools
import numpy as np
import ml_dtypes

import concourse.bass as bass
import concourse.mybir as mybir
import concourse.tile as tile
from concourse import bacc
from concourse.bass_utils import run_bass_kernel_spmd

# problem constants (hardcoded per contract)
B, S, H = 1, 2048, 4096
NQ, NKV, HD = 32, 8, 128
BLOCK = 128
NBLK = S // BLOCK          # 16
SINK_BLOCKS = 1
LOCAL_BLOCKS = 8
ROPE_BASE = 10000.0
N_CORES = 8
HQ = NQ // N_CORES         # 4 q heads per core
DQ = HQ * HD               # 512 q columns per core
SCALE = 1.0 / float(np.sqrt(HD))

KC = H // 128              # 32 contraction chunks for projections
NQUART = 4                 # S split into 4 quarters of 512 for projections
QW = S // NQUART           # 512
NCH = NBLK // 2            # 8 attention chunks of 256 queries

F32 = mybir.dt.float32
BF16 = mybir.dt.bfloat16

# Opt-in profiling plumbing (off by default; harness never touches these).
TRACE = False
TRACE_KW: dict = {}
LAST_RESULTS = None


def _pair_js(i: int) -> list[int]:
    """Key blocks contributing to query pair i (blocks 2i, 2i+1)."""
    return sorted(set([0]) | set(range(max(0, 2 * i - 7), 2 * i + 2)))


def build_nc(rs: bool = True, repeat: int = 1):
    nc = bacc.Bacc(
        "TRN2", target_bir_lowering=False, debug=False, num_devices=N_CORES
    )
    hidT = nc.dram_tensor("hidT", [H, S], BF16, kind="ExternalInput").ap()
    wq = nc.dram_tensor("wq", [H, DQ], BF16, kind="ExternalInput").ap()
    wk = nc.dram_tensor("wk", [H, HD], BF16, kind="ExternalInput").ap()
    wv = nc.dram_tensor("wv", [H, HD], BF16, kind="ExternalInput").ap()
    wo = nc.dram_tensor("wo", [H, DQ], BF16, kind="ExternalInput").ap()
    cosF = nc.dram_tensor("cosF", [128, S], F32, kind="ExternalInput").ap()
    sinS = nc.dram_tensor("sinS", [128, S], F32, kind="ExternalInput").ap()
    tri = nc.dram_tensor("tri", [128, 128], BF16, kind="ExternalInput").ap()
    eye = nc.dram_tensor("eye", [128, 128], F32, kind="ExternalInput").ap()
    perm = nc.dram_tensor("perm", [128, 128], BF16, kind="ExternalInput").ap()
    onescol = nc.dram_tensor("onescol", [128, 1], BF16, kind="ExternalInput").ap()
    maskA = nc.dram_tensor("maskA", [128, 256], BF16, kind="ExternalInput").ap()
    out = nc.dram_tensor("out", [S, DQ], F32, kind="ExternalOutput").ap()

    with tile.TileContext(nc) as tc:
      for rep in range(repeat):
        with (
            tc.tile_pool(name=f"persist{rep}", bufs=1) as pp,
            tc.tile_pool(name=f"dram{rep}", bufs=1, space="DRAM") as dramp,
            tc.tile_pool(name=f"sp{rep}", bufs=1) as sp,
            tc.tile_pool(name=f"stream{rep}", bufs=3) as stp,
            tc.tile_pool(name=f"e_sb{rep}", bufs=3) as ep,
            tc.tile_pool(name=f"att_sb{rep}", bufs=2) as asb,
            tc.tile_pool(name=f"ev_sb{rep}", bufs=3) as evp,
        ):
            qTr = [
                [
                    pp.tile([128, QW], BF16, tag=f"qTr{h}_{nq}", name=f"qTr{h}_{nq}")
                    for nq in range(NQUART)
                ]
                for h in range(HQ)
            ]
            kTr = [
                pp.tile([128, QW], BF16, tag=f"kTr{nq}", name=f"kTr{nq}")
                for nq in range(NQUART)
            ]
            vNat = [
                pp.tile([128, QW], BF16, tag=f"vNat{nq}", name=f"vNat{nq}")
                for nq in range(NQUART)
            ]
            tri_sb = pp.tile([128, 128], BF16, tag="tri", name="tri_sb")
            maskA_sb = pp.tile([128, 256], BF16, tag="maskA", name="maskA_sb")
            eye_sb = pp.tile([128, 128], F32, tag="eye", name="eye_sb")
            perm_sb = pp.tile([128, 128], BF16, tag="perm", name="perm_sb")
            ones_sb = pp.tile([128, 1], BF16, tag="ones", name="ones_sb")
            nc.sync.dma_start(tri_sb[:], tri[:])
            nc.sync.dma_start(maskA_sb[:], maskA[:])
            nc.sync.dma_start(eye_sb[:], eye[:])
            nc.sync.dma_start(perm_sb[:], perm[:])
            nc.sync.dma_start(ones_sb[:], onescol[:])

            # CC-stream warmup collective, issued first on gpsimd so the
            # ~50us NRT stream init overlaps the first projection quarter.
            if rs:
                warm_in = dramp.tile([16, 16], BF16, tag="warm_in", name="warm_in")
                warm_out = dramp.tile(
                    [128, 16], BF16, tag="warm_out", name="warm_out",
                    addr_space="Shared",
                )
                warm_src = pp.tile([16, 16], BF16, tag="warm_src", name="warm_src")
                nc.vector.memset(warm_src[:], 0.0)
                nc.gpsimd.dma_start(warm_in[:], warm_src[:])
                nc.gpsimd.collective_compute(
                    "AllGather",
                    mybir.AluOpType.bypass,
                    replica_groups=[list(range(N_CORES))],
                    ins=[warm_in.opt()],
                    outs=[warm_out.opt()],
                    unique_tensors="Yes",
                )

            # weight prefetch: qkv weights issue on SP in consumption order
            # (interleaved per chunk, pacing the quarter-0 matmuls); wo on
            # gpsimd (needed only from the second slot on).
            wo_t = []
            for c in range(KC):
                tw = sp.tile([128, DQ], BF16, tag=f"woc{c}", name=f"woc{c}")
                nc.gpsimd.dma_start(tw[:], wo[c * 128 : (c + 1) * 128, :])
                wo_t.append(tw)
            wq_t, wk_t, wv_t = [], [], []
            for c in range(KC):
                crow = slice(c * 128, (c + 1) * 128)
                tq = sp.tile([128, DQ], BF16, tag=f"wqc{c}", name=f"wqc{c}")
                tk = sp.tile([128, HD], BF16, tag=f"wkc{c}", name=f"wkc{c}")
                tv = sp.tile([128, HD], BF16, tag=f"wvc{c}", name=f"wvc{c}")
                nc.sync.dma_start(tq[:], wq[crow, :])
                nc.sync.dma_start(tk[:], wk[crow, :])
                nc.sync.dma_start(tv[:], wv[crow, :])
                wq_t.append(tq)
                wk_t.append(tk)
                wv_t.append(tv)

            vT = sp.tile([128, S], F32, tag="vT", name="vT")
            ag_ins = [
                dramp.tile([DQ, 256], BF16, tag=f"agin{c}", name=f"agin{c}")
                for c in range(NCH)
            ]
            ag_outs = [
                dramp.tile(
                    [H, 256], BF16, tag=f"agout{c}", name=f"agout{c}",
                    addr_space="Shared",
                )
                for c in range(NCH)
            ]

            def attn_chunk(i: int, apool):
                q0 = i * 256
                js = _pair_js(i)
                L = len(js)
                for h in range(HQ):
                    e_t = ep.tile([128, L * 256], BF16, tag="e", name="e_t")
                    oT = apool.tile([128, 256], F32, tag="oT", name="oT", bufs=2)
                    sm = apool.tile([1, 256], F32, tag="sm", name="sm", bufs=1)

                    spans = []
                    for j in js:
                        left = (j == 0) or (j <= 2 * i <= j + 7)
                        right = (j == 0) or (j <= 2 * i + 1 <= j + 7)
                        qs = q0 if left else q0 + 128
                        qe = q0 + 256 if right else q0 + 128
                        spans.append((qs, qe))

                    def score(idx: int):
                        j = js[idx]
                        qs, qe = spans[idx]
                        w = qe - qs
                        ecols = slice(idx * 256, idx * 256 + w)
                        s_ps = apool.tile(
                            [128, 256], F32, tag="sps", name="s_ps", bufs=3
                        )
                        kq_, kc_ = j // 4, (j % 4) * 128
                        qq_ = qs // QW
                        nc.tensor.matmul(
                            s_ps[:, 0:w],
                            kTr[kq_][:, kc_ : kc_ + 128],
                            qTr[h][qq_][:, qs - qq_ * QW : qe - qq_ * QW],
                            start=True,
                            stop=True,
                        )
                        nc.scalar.activation(
                            e_t[:, ecols],
                            s_ps[:, 0:w],
                            mybir.ActivationFunctionType.Exp,
                            scale=SCALE,
                        )
                        if j == 2 * i:
                            nc.vector.tensor_mul(
                                e_t[:, ecols], e_t[:, ecols], maskA_sb[:]
                            )
                        elif j == 2 * i + 1:
                            nc.vector.tensor_mul(
                                e_t[:, ecols], e_t[:, ecols], tri_sb[:]
                            )

                    def av(idx: int):
                        j = js[idx]
                        qs, qe = spans[idx]
                        w = qe - qs
                        ecols = slice(idx * 256, idx * 256 + w)
                        st, sp_ = (idx == 0), (idx == L - 1)
                        nc.tensor.matmul(
                            oT[:, qs - q0 : qe - q0],
                            vNat[j // 4][:, (j % 4) * 128 : (j % 4 + 1) * 128],
                            e_t[:, ecols],
                            start=st,
                            stop=sp_,
                        )
                        nc.tensor.matmul(
                            sm[:, qs - q0 : qe - q0],
                            ones_sb[:],
                            e_t[:, ecols],
                            start=st,
                            stop=sp_,
                        )

                    score(0)
                    if L > 1:
                        score(1)
                    for idx in range(L):
                        if idx + 2 < L:
                            score(idx + 2)
                        av(idx)

                    r_sb = asb.tile([1, 256], F32, tag="r", name="r_sb")
                    nc.vector.reciprocal_approx_fast(r_sb[:], sm[:])
                    rb = asb.tile([128, 256], F32, tag="rb", name="rb")
                    nc.gpsimd.partition_broadcast(rb[:], r_sb[:])
                    at_c = asb.tile([128, 256], BF16, tag=f"at{h}", name=f"at{h}")
                    nc.vector.tensor_mul(at_c[:], oT[:], rb[:])
                    nc.sync.dma_start(
                        ag_ins[i][h * 128 : (h + 1) * 128, :], at_c[:]
                    )

                if rs:
                    nc.gpsimd.collective_compute(
                        "AllGather",
                        mybir.AluOpType.bypass,
                        replica_groups=[list(range(N_CORES))],
                        ins=[ag_ins[i].opt()],
                        outs=[ag_outs[i].opt()],
                        unique_tensors="Yes",
                    )
                else:
                    nc.sync.dma_start(ag_outs[i][0:DQ, :], ag_ins[i][:])

            def oproj_chunk(i: int, oppool):
                q0 = i * 256
                ps01 = [
                    oppool.tile(
                        [128, 512], F32, tag=f"op{sb}", name=f"op{sb}", bufs=1
                    )
                    for sb in range(2)
                ]
                # ag loads issue on gpsimd: they depend on collectives, and a
                # scheduler-hoisted one on the SP queue head-of-line blocks
                # the hid stream behind an in-flight AllGather
                for c in range(KC):
                    ag_sb = evp.tile(
                        [128, 256], BF16, tag="ag_sb", name="ag_sb", bufs=6
                    )
                    nc.gpsimd.dma_start(
                        ag_sb[:], ag_outs[i][c * 128 : (c + 1) * 128, :]
                    )
                    for sb in range(2):
                        nc.tensor.matmul(
                            ps01[sb][:],
                            ag_sb[:, sb * 128 : (sb + 1) * 128],
                            wo_t[c][:],
                            start=(c == 0),
                            stop=(c == KC - 1),
                        )
                for sb in range(2):
                    ev = evp.tile([128, 512], F32, tag="ev", name="ev")
                    nc.scalar.copy(ev[:], ps01[sb][:])
                    nc.gpsimd.dma_start(
                        out[q0 + sb * 128 : q0 + (sb + 1) * 128, :], ev[:]
                    )

            for nq in range(NQUART):
                ncols = slice(nq * QW, (nq + 1) * QW)
                with tc.tile_pool(
                    name=f"qps{rep}_{nq}", bufs=1, space="PSUM"
                ) as qpool:
                    ps_q = [
                        qpool.tile([128, QW], F32, tag=f"psq{h}", name=f"psq{h}")
                        for h in range(HQ)
                    ]
                    ps_k = qpool.tile([128, QW], F32, tag="psk", name="ps_k")
                    ps_v = qpool.tile([128, QW], F32, tag="psv", name="ps_v")
                    # quarter 0's hid stream issues on ACT so it doesn't sit
                    # behind the 96 weight DMAs on the SP queue
                    hid_eng = nc.scalar if nq == 0 else nc.sync
                    for c in range(KC):
                        crow = slice(c * 128, (c + 1) * 128)
                        hid_c = stp.tile([128, QW], BF16, tag="hid", name="hid_c")
                        hid_eng.dma_start(hid_c[:], hidT[crow, ncols])
                        st, sp_ = (c == 0), (c == KC - 1)
                        for h in range(HQ):
                            nc.tensor.matmul(
                                ps_q[h][:],
                                wq_t[c][:, h * HD : (h + 1) * HD],
                                hid_c[:],
                                start=st,
                                stop=sp_,
                            )
                        nc.tensor.matmul(
                            ps_k[:], wk_t[c][:], hid_c[:], start=st, stop=sp_
                        )
                        nc.tensor.matmul(
                            ps_v[:], wv_t[c][:], hid_c[:], start=st, stop=sp_
                        )

                    cos_sb = stp.tile([128, QW], F32, tag="cos", name="cos_sb", bufs=2)
                    sin_sb = stp.tile([128, QW], F32, tag="sin", name="sin_sb", bufs=2)
                    nc.sync.dma_start(cos_sb[:], cosF[:, ncols])
                    nc.sync.dma_start(sin_sb[:], sinS[:, ncols])

                    # Evacuate PSUM on ACT (bf16 raws) and RoPE on DVE,
                    # grouped per tensor with q0 first then k: the first
                    # attention chunk needs qTr[0] and the fresh kTr
                    # earliest. The rotate-half swap runs as a PE
                    # permutation matmul into PSUM (PE is idle here and it
                    # beats a DMA round trip by ~3us of latency).
                    rope_list = [(ps_q[0], qTr[0][nq], "q0"), (ps_k, kTr[nq], "k")]
                    rope_list += [
                        (ps_q[h], qTr[h][nq], f"q{h}") for h in range(1, HQ)
                    ]
                    with tc.tile_pool(
                        name=f"trp{rep}_{nq}", bufs=2, space="PSUM"
                    ) as trpool:
                        for ps_x, dstT, tag in rope_list:
                            raw = sp.tile(
                                [128, QW], BF16, tag=f"raw{tag}", name=f"raw{tag}"
                            )
                            nc.scalar.copy(raw[:], ps_x[:])
                            swp = trpool.tile([128, QW], F32, tag="tr", name="swp")
                            nc.tensor.matmul(
                                swp[:], perm_sb[:], raw[:], start=True, stop=True
                            )
                            t1 = sp.tile([128, QW], F32, tag=f"t1{tag}", name=f"t1{tag}")
                            nc.vector.tensor_mul(t1[:], raw[:], cos_sb[:])
                            t2 = sp.tile([128, QW], F32, tag=f"t2{tag}", name=f"t2{tag}")
                            nc.vector.tensor_mul(t2[:], swp[:], sin_sb[:])
                            nc.vector.tensor_add(dstT[:], t1[:], t2[:])
                        nc.scalar.copy(vT[:, ncols], ps_v[:])

                        # V natural blocks for this quarter (4 transposes)
                        for jb in range(nq * QW // 128, (nq + 1) * QW // 128):
                            bcols = slice(jb * 128, (jb + 1) * 128)
                            lcols = slice((jb % 4) * 128, (jb % 4 + 1) * 128)
                            tr = trpool.tile([128, QW], F32, tag="tr", name="tr")
                            nc.tensor.transpose(
                                tr[:, 0:128], vT[:, bcols], eye_sb[:]
                            )
                            nc.scalar.copy(vNat[nq][:, lcols], tr[:, 0:128])

                # attention chunks unlocked by this quarter; o_proj lags by
                # 2 chunks so each AllGather hides under subsequent PE work.
                # The last slot runs chunk 7 before 6 so AG7 is covered by
                # attn(6) + o_proj work instead of sticking out as a tail.
                # attention chunks unlocked by this quarter; o_proj lags by
                # 2 chunks so each AllGather hides under subsequent PE work.
                # The last slot runs chunk 7 before 6 so AG7 is covered by
                # attn(6) + o_proj work instead of sticking out as a tail.
                ca, cb = (2 * nq, 2 * nq + 1) if nq < 3 else (7, 6)
                with tc.tile_pool(
                    name=f"aps{rep}_{nq}", bufs=1, space="PSUM"
                ) as apool:
                    attn_chunk(ca, apool)
                    attn_chunk(cb, apool)
                    # both lagged o_projs after both attention chunks: all
                    # broadcasts (AG critical path) precede all ag loads on
                    # the gpsimd queue, and the loads' AllGathers are 1.5+
                    # slots old so they never head-of-line block anything
                    for lag in (2 * nq - 2, 2 * nq - 1):
                        if lag >= 0:
                            with tc.tile_pool(
                                name=f"ops{rep}_{nq}_{lag}", bufs=1, space="PSUM"
                            ) as oppool:
                                oproj_chunk(lag, oppool)

            with tc.tile_pool(
                name=f"ops{rep}_t6", bufs=1, space="PSUM"
            ) as oppool:
                oproj_chunk(NCH - 2, oppool)
            with tc.tile_pool(
                name=f"ops{rep}_t7", bufs=1, space="PSUM"
            ) as oppool:
                oproj_chunk(NCH - 1, oppool)
    nc.compile()
    return nc


@functools.lru_cache(maxsize=1)
def _cached_nc():
    return build_nc(rs=True)


def _tables():
    pos = np.arange(S, dtype=np.float64)
    inv = 1.0 / (ROPE_BASE ** (np.arange(0, HD, 2, dtype=np.float64) / HD))  # [64]
    f = inv[:, None] * pos[None, :]                   # [64, S]
    cos = np.cos(f).astype(np.float32)
    sin = np.sin(f).astype(np.float32)
    cosF = np.concatenate([cos, cos], axis=0)         # [128, S]
    sinS = np.concatenate([-sin, sin], axis=0)        # [128, S]
    k_idx = np.arange(128)[:, None]
    q_idx = np.arange(128)[None, :]
    tri = (k_idx <= q_idx).astype(np.float32)         # [k, q] causal in-block
    eye = np.eye(128, dtype=np.float32)
    maskA = np.concatenate([tri, np.ones((128, 128), np.float32)], axis=1)
    # rotate-half permutation: out[m] = in[(m+64) % 128] via P^T @ in
    permM = np.zeros((128, 128), np.float32)
    permM[(np.arange(128) + 64) % 128, np.arange(128)] = 1.0
    return cosF, sinS, tri, eye, maskA, permM


def _bf16(x: np.ndarray) -> np.ndarray:
    return np.ascontiguousarray(x).astype(ml_dtypes.bfloat16)


def kernel(hidden_states, wq, wk, wv, wo):
    nc = _cached_nc()
    hidT = _bf16(np.asarray(hidden_states, dtype=np.float32).reshape(S, H).T)
    cosF, sinS, tri, eye, maskA, permM = _tables()
    in_maps = []
    for c in range(N_CORES):
        in_maps.append(
            {
                "hidT": hidT,
                "wq": _bf16(wq[:, c * DQ : (c + 1) * DQ]),
                "wk": _bf16(wk[:, c * HD : (c + 1) * HD]),
                "wv": _bf16(wv[:, c * HD : (c + 1) * HD]),
                "wo": _bf16(wo[:, c * DQ : (c + 1) * DQ]),
                "cosF": cosF,
                "sinS": sinS,
                "tri": _bf16(tri),
                "eye": eye,
                "perm": _bf16(permM),
                "onescol": np.ones((128, 1), dtype=ml_dtypes.bfloat16),
                "maskA": _bf16(maskA),
            }
        )
    kw = dict(trace=True, **TRACE_KW) if TRACE else {}
    res = run_bass_kernel_spmd(nc, in_maps, core_ids=list(range(N_CORES)), **kw)
    global LAST_RESULTS
    LAST_RESULTS = res
    full = np.concatenate(
        [res.results[r]["out"] for r in range(N_CORES)], axis=1
    )
    return full.reshape(B, S, H)


# revision 31
# speedup vs baseline: 1.1684x; 1.0362x over previous
"""Trainium2 Bass kernel for nn_LlamaAttention_48816598286577.

Llama attention with block-streaming sparse mask (sink=1 block, local
window=8 blocks, BLOCK=128), B=1 S=2048 H=4096, 32 q heads / 8 kv heads,
head_dim 128, non-interleaved RoPE.

Sharding: tensor-parallel over heads across 8 cores (4 q heads + 1 kv
head per core). All matmul operands are bf16 (f32r measured ~2x slower
per row and ~4x slower LDWEIGHTS on HW); accumulation is f32 in PSUM.

Schedule: the S=2048 sequence is processed in 4 projection quarters.
After each quarter's QKV+RoPE, the two 256-query attention chunks it
unlocks run immediately, each followed by its bf16 AllGather; o_proj
for chunk i runs two chunks later, so every AllGather hides under
attention + o_proj + next-quarter PE work. DMA descriptor generation is
spread across the Sync/Scalar/Vector/GpSimd queues to avoid the
in-order SP queue serializing issue (565ns each).
"""

import functools
import numpy as np
import ml_dtypes

import concourse.bass as bass
import concourse.mybir as mybir
import concourse.tile as tile
from concourse import bacc
from concourse.bass_utils import run_bass_kernel_spmd

# problem constants (hardcoded per contract)
B, S, H = 1, 2048, 4096
NQ, NKV, HD = 32, 8, 128
BLOCK = 128
NBLK = S // BLOCK          # 16
SINK_BLOCKS = 1
LOCAL_BLOCKS = 8
ROPE_BASE = 10000.0
N_CORES = 8
HQ = NQ // N_CORES         # 4 q heads per core
DQ = HQ * HD               # 512 q columns per core
SCALE = 1.0 / float(np.sqrt(HD))

KC = H // 128              # 32 contraction chunks for projections
NQUART = 4                 # S split into 4 quarters of 512 for projections
QW = S // NQUART           # 512
NCH = NBLK // 2            # 8 attention chunks of 256 queries

F32 = mybir.dt.float32
BF16 = mybir.dt.bfloat16

# Opt-in profiling plumbing (off by default; harness never touches these).
TRACE = False
TRACE_KW: dict = {}
LAST_RESULTS = None


def _pair_js(i: int) -> list[int]:
    """Key blocks contributing to query pair i (blocks 2i, 2i+1)."""
    return sorted(set([0]) | set(range(max(0, 2 * i - 7), 2 * i + 2)))


def build_nc(rs: bool = True, repeat: int = 1):
    nc = bacc.Bacc(
        "TRN2", target_bir_lowering=False, debug=False, num_devices=N_CORES
    )
    hidT = nc.dram_tensor("hidT", [H, S], BF16, kind="ExternalInput").ap()
    wq = nc.dram_tensor("wq", [H, DQ], BF16, kind="ExternalInput").ap()
    wk = nc.dram_tensor("wk", [H, HD], BF16, kind="ExternalInput").ap()
    wv = nc.dram_tensor("wv", [H, HD], BF16, kind="ExternalInput").ap()
    wo = nc.dram_tensor("wo", [H, DQ], BF16, kind="ExternalInput").ap()
    cosF = nc.dram_tensor("cosF", [128, S], F32, kind="ExternalInput").ap()
    sinS = nc.dram_tensor("sinS", [128, S], F32, kind="ExternalInput").ap()
    tri = nc.dram_tensor("tri", [128, 128], BF16, kind="ExternalInput").ap()
    eye = nc.dram_tensor("eye", [128, 128], F32, kind="ExternalInput").ap()
    perm = nc.dram_tensor("perm", [128, 128], BF16, kind="ExternalInput").ap()
    onescol = nc.dram_tensor("onescol", [128, 1], BF16, kind="ExternalInput").ap()
    maskA = nc.dram_tensor("maskA", [128, 256], BF16, kind="ExternalInput").ap()
    out = nc.dram_tensor("out", [S, DQ], F32, kind="ExternalOutput").ap()

    with tile.TileContext(nc) as tc:
      for rep in range(repeat):
        with (
            tc.tile_pool(name=f"persist{rep}", bufs=1) as pp,
            tc.tile_pool(name=f"dram{rep}", bufs=1, space="DRAM") as dramp,
            tc.tile_pool(name=f"sp{rep}", bufs=1) as sp,
            tc.tile_pool(name=f"stream{rep}", bufs=4) as stp,
            tc.tile_pool(name=f"e_sb{rep}", bufs=4) as ep,
            tc.tile_pool(name=f"att_sb{rep}", bufs=2) as asb,
            tc.tile_pool(name=f"ev_sb{rep}", bufs=3) as evp,
        ):
            qTr = [
                [
                    pp.tile([128, QW], BF16, tag=f"qTr{h}_{nq}", name=f"qTr{h}_{nq}")
                    for nq in range(NQUART)
                ]
                for h in range(HQ)
            ]
            kTr = [
                pp.tile([128, QW], BF16, tag=f"kTr{nq}", name=f"kTr{nq}")
                for nq in range(NQUART)
            ]
            vNat = [
                pp.tile([128, QW], BF16, tag=f"vNat{nq}", name=f"vNat{nq}")
                for nq in range(NQUART)
            ]
            tri_sb = pp.tile([128, 128], BF16, tag="tri", name="tri_sb")
            maskA_sb = pp.tile([128, 256], BF16, tag="maskA", name="maskA_sb")
            eye_sb = pp.tile([128, 128], F32, tag="eye", name="eye_sb")
            perm_sb = pp.tile([128, 128], BF16, tag="perm", name="perm_sb")
            ones_sb = pp.tile([128, 1], BF16, tag="ones", name="ones_sb")
            # mask/table loads go on ACT: they're needed only from the first
            # attention chunk (~60us in), and keeping them off SP lets the
            # first projection chunk's weight DMAs issue immediately
            nc.scalar.dma_start(tri_sb[:], tri[:])
            nc.scalar.dma_start(maskA_sb[:], maskA[:])
            nc.scalar.dma_start(eye_sb[:], eye[:])
            nc.scalar.dma_start(perm_sb[:], perm[:])
            nc.scalar.dma_start(ones_sb[:], onescol[:])

            # CC-stream warmup collective, issued first on gpsimd so the
            # ~50us NRT stream init overlaps the first projection quarter.
            if rs:
                warm_in = dramp.tile([16, 16], BF16, tag="warm_in", name="warm_in")
                warm_out = dramp.tile(
                    [128, 16], BF16, tag="warm_out", name="warm_out",
                    addr_space="Shared",
                )
                warm_src = pp.tile([16, 16], BF16, tag="warm_src", name="warm_src")
                nc.vector.memset(warm_src[:], 0.0)
                nc.gpsimd.dma_start(warm_in[:], warm_src[:])
                nc.gpsimd.collective_compute(
                    "AllGather",
                    mybir.AluOpType.bypass,
                    replica_groups=[list(range(N_CORES))],
                    ins=[warm_in.opt()],
                    outs=[warm_out.opt()],
                    unique_tensors="Yes",
                )

            # weight prefetch: qkv weights issue on SP in consumption order
            # (interleaved per chunk, pacing the quarter-0 matmuls); wo on
            # gpsimd (needed only from the second slot on).
            wo_t = []
            for c in range(KC):
                tw = sp.tile([128, DQ], BF16, tag=f"woc{c}", name=f"woc{c}")
                nc.gpsimd.dma_start(tw[:], wo[c * 128 : (c + 1) * 128, :])
                wo_t.append(tw)
            wq_t, wk_t, wv_t = [], [], []
            for c in range(KC):
                crow = slice(c * 128, (c + 1) * 128)
                tq = sp.tile([128, DQ], BF16, tag=f"wqc{c}", name=f"wqc{c}")
                tk = sp.tile([128, HD], BF16, tag=f"wkc{c}", name=f"wkc{c}")
                tv = sp.tile([128, HD], BF16, tag=f"wvc{c}", name=f"wvc{c}")
                nc.sync.dma_start(tq[:], wq[crow, :])
                nc.sync.dma_start(tk[:], wk[crow, :])
                nc.sync.dma_start(tv[:], wv[crow, :])
                wq_t.append(tq)
                wk_t.append(tk)
                wv_t.append(tv)

            vT = sp.tile([128, S], F32, tag="vT", name="vT")
            ag_ins = [
                dramp.tile([DQ, 256], BF16, tag=f"agin{c}", name=f"agin{c}")
                for c in range(NCH)
            ]
            ag_outs = [
                dramp.tile(
                    [H, 256], BF16, tag=f"agout{c}", name=f"agout{c}",
                    addr_space="Shared",
                )
                for c in range(NCH)
            ]

            def attn_chunk(i: int, apool):
                q0 = i * 256
                js = _pair_js(i)
                L = len(js)
                for h in range(HQ):
                    e_t = ep.tile([128, L * 256], BF16, tag="e", name="e_t")
                    oT = apool.tile([128, 256], F32, tag="oT", name="oT", bufs=2)
                    sm = apool.tile([1, 256], F32, tag="sm", name="sm", bufs=1)

                    spans = []
                    for j in js:
                        left = (j == 0) or (j <= 2 * i <= j + 7)
                        right = (j == 0) or (j <= 2 * i + 1 <= j + 7)
                        qs = q0 if left else q0 + 128
                        qe = q0 + 256 if right else q0 + 128
                        spans.append((qs, qe))

                    def score(idx: int):
                        j = js[idx]
                        qs, qe = spans[idx]
                        w = qe - qs
                        ecols = slice(idx * 256, idx * 256 + w)
                        s_ps = apool.tile(
                            [128, 256], F32, tag="sps", name="s_ps", bufs=3
                        )
                        kq_, kc_ = j // 4, (j % 4) * 128
                        qq_ = qs // QW
                        nc.tensor.matmul(
                            s_ps[:, 0:w],
                            kTr[kq_][:, kc_ : kc_ + 128],
                            qTr[h][qq_][:, qs - qq_ * QW : qe - qq_ * QW],
                            start=True,
                            stop=True,
                        )
                        nc.scalar.activation(
                            e_t[:, ecols],
                            s_ps[:, 0:w],
                            mybir.ActivationFunctionType.Exp,
                            scale=SCALE,
                        )
                        if j == 2 * i:
                            nc.vector.tensor_mul(
                                e_t[:, ecols], e_t[:, ecols], maskA_sb[:]
                            )
                        elif j == 2 * i + 1:
                            nc.vector.tensor_mul(
                                e_t[:, ecols], e_t[:, ecols], tri_sb[:]
                            )

                    def av(idx: int):
                        j = js[idx]
                        qs, qe = spans[idx]
                        w = qe - qs
                        ecols = slice(idx * 256, idx * 256 + w)
                        st, sp_ = (idx == 0), (idx == L - 1)
                        nc.tensor.matmul(
                            oT[:, qs - q0 : qe - q0],
                            vNat[j // 4][:, (j % 4) * 128 : (j % 4 + 1) * 128],
                            e_t[:, ecols],
                            start=st,
                            stop=sp_,
                        )
                        nc.tensor.matmul(
                            sm[:, qs - q0 : qe - q0],
                            ones_sb[:],
                            e_t[:, ecols],
                            start=st,
                            stop=sp_,
                        )

                    score(0)
                    if L > 1:
                        score(1)
                    for idx in range(L):
                        if idx + 2 < L:
                            score(idx + 2)
                        av(idx)

                    r_sb = asb.tile([1, 256], F32, tag="r", name="r_sb")
                    nc.vector.reciprocal_approx_fast(r_sb[:], sm[:])
                    rb = asb.tile([128, 256], F32, tag="rb", name="rb")
                    nc.gpsimd.partition_broadcast(rb[:], r_sb[:])
                    at_c = asb.tile([128, 256], BF16, tag=f"at{h}", name=f"at{h}")
                    nc.vector.tensor_mul(at_c[:], oT[:], rb[:])
                    nc.sync.dma_start(
                        ag_ins[i][h * 128 : (h + 1) * 128, :], at_c[:]
                    )

                if rs:
                    nc.gpsimd.collective_compute(
                        "AllGather",
                        mybir.AluOpType.bypass,
                        replica_groups=[list(range(N_CORES))],
                        ins=[ag_ins[i].opt()],
                        outs=[ag_outs[i].opt()],
                        unique_tensors="Yes",
                    )
                else:
                    nc.sync.dma_start(ag_outs[i][0:DQ, :], ag_ins[i][:])

            def oproj_chunk(i: int, oppool):
                q0 = i * 256
                ps01 = [
                    oppool.tile(
                        [128, 512], F32, tag=f"op{sb}", name=f"op{sb}", bufs=1
                    )
                    for sb in range(2)
                ]
                # ag loads issue on gpsimd: they depend on collectives, and a
                # scheduler-hoisted one on the SP queue head-of-line blocks
                # the hid stream behind an in-flight AllGather
                for c in range(KC):
                    ag_sb = evp.tile(
                        [128, 256], BF16, tag="ag_sb", name="ag_sb", bufs=8
                    )
                    nc.gpsimd.dma_start(
                        ag_sb[:], ag_outs[i][c * 128 : (c + 1) * 128, :]
                    )
                    for sb in range(2):
                        nc.tensor.matmul(
                            ps01[sb][:],
                            ag_sb[:, sb * 128 : (sb + 1) * 128],
                            wo_t[c][:],
                            start=(c == 0),
                            stop=(c == KC - 1),
                        )
                for sb in range(2):
                    ev = evp.tile([128, 512], F32, tag="ev", name="ev")
                    nc.scalar.copy(ev[:], ps01[sb][:])
                    nc.gpsimd.dma_start(
                        out[q0 + sb * 128 : q0 + (sb + 1) * 128, :], ev[:]
                    )

            for nq in range(NQUART):
                ncols = slice(nq * QW, (nq + 1) * QW)
                with tc.tile_pool(
                    name=f"qps{rep}_{nq}", bufs=1, space="PSUM"
                ) as qpool:
                    ps_q = [
                        qpool.tile([128, QW], F32, tag=f"psq{h}", name=f"psq{h}")
                        for h in range(HQ)
                    ]
                    ps_k = qpool.tile([128, QW], F32, tag="psk", name="ps_k")
                    ps_v = qpool.tile([128, QW], F32, tag="psv", name="ps_v")
                    # quarter 0's hid stream issues on ACT so it doesn't sit
                    # behind the 96 weight DMAs on the SP queue
                    hid_eng = nc.scalar if nq == 0 else nc.sync
                    for c in range(KC):
                        crow = slice(c * 128, (c + 1) * 128)
                        hid_c = stp.tile([128, QW], BF16, tag="hid", name="hid_c")
                        hid_eng.dma_start(hid_c[:], hidT[crow, ncols])
                        st, sp_ = (c == 0), (c == KC - 1)
                        for h in range(HQ):
                            nc.tensor.matmul(
                                ps_q[h][:],
                                wq_t[c][:, h * HD : (h + 1) * HD],
                                hid_c[:],
                                start=st,
                                stop=sp_,
                            )
                        nc.tensor.matmul(
                            ps_k[:], wk_t[c][:], hid_c[:], start=st, stop=sp_
                        )
                        nc.tensor.matmul(
                            ps_v[:], wv_t[c][:], hid_c[:], start=st, stop=sp_
                        )

                    cos_sb = stp.tile([128, QW], F32, tag="cos", name="cos_sb", bufs=2)
                    sin_sb = stp.tile([128, QW], F32, tag="sin", name="sin_sb", bufs=2)
                    nc.sync.dma_start(cos_sb[:], cosF[:, ncols])
                    nc.sync.dma_start(sin_sb[:], sinS[:, ncols])

                    # Evacuate PSUM on ACT (bf16 raws) and RoPE on DVE,
                    # grouped per tensor with q0 first then k: the first
                    # attention chunk needs qTr[0] and the fresh kTr
                    # earliest. The rotate-half swap runs as a PE
                    # permutation matmul into PSUM (PE is idle here and it
                    # beats a DMA round trip by ~3us of latency).
                    rope_list = [(ps_q[0], qTr[0][nq], "q0"), (ps_k, kTr[nq], "k")]
                    rope_list += [
                        (ps_q[h], qTr[h][nq], f"q{h}") for h in range(1, HQ)
                    ]
                    with tc.tile_pool(
                        name=f"trp{rep}_{nq}", bufs=2, space="PSUM"
                    ) as trpool:
                        for ps_x, dstT, tag in rope_list:
                            raw = sp.tile(
                                [128, QW], BF16, tag=f"raw{tag}", name=f"raw{tag}"
                            )
                            nc.scalar.copy(raw[:], ps_x[:])
                            swp = trpool.tile([128, QW], F32, tag="tr", name="swp")
                            nc.tensor.matmul(
                                swp[:], perm_sb[:], raw[:], start=True, stop=True
                            )
                            t1 = sp.tile([128, QW], F32, tag=f"t1{tag}", name=f"t1{tag}")
                            nc.vector.tensor_mul(t1[:], raw[:], cos_sb[:])
                            t2 = sp.tile([128, QW], F32, tag=f"t2{tag}", name=f"t2{tag}")
                            nc.vector.tensor_mul(t2[:], swp[:], sin_sb[:])
                            nc.vector.tensor_add(dstT[:], t1[:], t2[:])
                        nc.scalar.copy(vT[:, ncols], ps_v[:])

                        # V natural blocks for this quarter (4 transposes)
                        for jb in range(nq * QW // 128, (nq + 1) * QW // 128):
                            bcols = slice(jb * 128, (jb + 1) * 128)
                            lcols = slice((jb % 4) * 128, (jb % 4 + 1) * 128)
                            tr = trpool.tile([128, QW], F32, tag="tr", name="tr")
                            nc.tensor.transpose(
                                tr[:, 0:128], vT[:, bcols], eye_sb[:]
                            )
                            nc.scalar.copy(vNat[nq][:, lcols], tr[:, 0:128])

                # attention chunks unlocked by this quarter; o_proj lags by
                # 2 chunks so each AllGather hides under subsequent PE work.
                # The last slot runs chunk 7 before 6 so AG7 is covered by
                # attn(6) + o_proj work instead of sticking out as a tail.
                # attention chunks unlocked by this quarter; o_proj lags by
                # 2 chunks so each AllGather hides under subsequent PE work.
                # The last slot runs chunk 7 before 6 so AG7 is covered by
                # attn(6) + o_proj work instead of sticking out as a tail.
                ca, cb = (2 * nq, 2 * nq + 1) if nq < 3 else (7, 6)
                with tc.tile_pool(
                    name=f"aps{rep}_{nq}", bufs=1, space="PSUM"
                ) as apool:
                    attn_chunk(ca, apool)
                    attn_chunk(cb, apool)
                    # both lagged o_projs after both attention chunks: all
                    # broadcasts (AG critical path) precede all ag loads on
                    # the gpsimd queue, and the loads' AllGathers are 1.5+
                    # slots old so they never head-of-line block anything
                    for lag in (2 * nq - 2, 2 * nq - 1):
                        if lag >= 0:
                            with tc.tile_pool(
                                name=f"ops{rep}_{nq}_{lag}", bufs=1, space="PSUM"
                            ) as oppool:
                                oproj_chunk(lag, oppool)

            with tc.tile_pool(
                name=f"ops{rep}_t6", bufs=1, space="PSUM"
            ) as oppool:
                oproj_chunk(NCH - 2, oppool)
            with tc.tile_pool(
                name=f"ops{rep}_t7", bufs=1, space="PSUM"
            ) as oppool:
                oproj_chunk(NCH - 1, oppool)
    nc.compile()
    return nc


@functools.lru_cache(maxsize=1)
def _cached_nc():
    return build_nc(rs=True)


def _tables():
    pos = np.arange(S, dtype=np.float64)
    inv = 1.0 / (ROPE_BASE ** (np.arange(0, HD, 2, dtype=np.float64) / HD))  # [64]
    f = inv[:, None] * pos[None, :]                   # [64, S]
    cos = np.cos(f).astype(np.float32)
    sin = np.sin(f).astype(np.float32)
    cosF = np.concatenate([cos, cos], axis=0)         # [128, S]
    sinS = np.concatenate([-sin, sin], axis=0)        # [128, S]
    k_idx = np.arange(128)[:, None]
    q_idx = np.arange(128)[None, :]
    tri = (k_idx <= q_idx).astype(np.float32)         # [k, q] causal in-block
    eye = np.eye(128, dtype=np.float32)
    maskA = np.concatenate([tri, np.ones((128, 128), np.float32)], axis=1)
    # rotate-half permutation: out[m] = in[(m+64) % 128] via P^T @ in
    permM = np.zeros((128, 128), np.float32)
    permM[(np.arange(128) + 64) % 128, np.arange(128)] = 1.0
    return cosF, sinS, tri, eye, maskA, permM


def _bf16(x: np.ndarray) -> np.ndarray:
    return np.ascontiguousarray(x).astype(ml_dtypes.bfloat16)


def kernel(hidden_states, wq, wk, wv, wo):
    nc = _cached_nc()
    hidT = _bf16(np.asarray(hidden_states, dtype=np.float32).reshape(S, H).T)
    cosF, sinS, tri, eye, maskA, permM = _tables()
    in_maps = []
    for c in range(N_CORES):
        in_maps.append(
            {
                "hidT": hidT,
                "wq": _bf16(wq[:, c * DQ : (c + 1) * DQ]),
                "wk": _bf16(wk[:, c * HD : (c + 1) * HD]),
                "wv": _bf16(wv[:, c * HD : (c + 1) * HD]),
                "wo": _bf16(wo[:, c * DQ : (c + 1) * DQ]),
                "cosF": cosF,
                "sinS": sinS,
                "tri": _bf16(tri),
                "eye": eye,
                "perm": _bf16(permM),
                "onescol": np.ones((128, 1), dtype=ml_dtypes.bfloat16),
                "maskA": _bf16(maskA),
            }
        )
    kw = dict(trace=True, **TRACE_KW) if TRACE else {}
    res = run_bass_kernel_spmd(nc, in_maps, core_ids=list(range(N_CORES)), **kw)
    global LAST_RESULTS
    LAST_RESULTS = res
    full = np.concatenate(
        [res.results[r]["out"] for r in range(N_CORES)], axis=1
    )
    return full.reshape(B, S, H)


# revision 33
# speedup vs baseline: 1.1784x; 1.0085x over previous
"""Trainium2 Bass kernel for nn_LlamaAttention_48816598286577.

Llama attention with block-streaming sparse mask (sink=1 block, local
window=8 blocks, BLOCK=128), B=1 S=2048 H=4096, 32 q heads / 8 kv heads,
head_dim 128, non-interleaved RoPE.

Sharding: tensor-parallel over heads across 8 cores (4 q heads + 1 kv
head per core). All matmul operands are bf16 (f32r measured ~2x slower
per row and ~4x slower LDWEIGHTS on HW); accumulation is f32 in PSUM.

Schedule: the S=2048 sequence is processed in 4 projection quarters.
After each quarter's QKV+RoPE, the two 256-query attention chunks it
unlocks run immediately, each followed by its bf16 AllGather; o_proj
for chunk i runs two chunks later, so every AllGather hides under
attention + o_proj + next-quarter PE work. DMA descriptor generation is
spread across the Sync/Scalar/Vector/GpSimd queues to avoid the
in-order SP queue serializing issue (565ns each).
"""

import functools
import numpy as np
import ml_dtypes

import concourse.bass as bass
import concourse.mybir as mybir
import concourse.tile as tile
from concourse import bacc
from concourse.bass_utils import run_bass_kernel_spmd

# problem constants (hardcoded per contract)
B, S, H = 1, 2048, 4096
NQ, NKV, HD = 32, 8, 128
BLOCK = 128
NBLK = S // BLOCK          # 16
SINK_BLOCKS = 1
LOCAL_BLOCKS = 8
ROPE_BASE = 10000.0
N_CORES = 8
HQ = NQ // N_CORES         # 4 q heads per core
DQ = HQ * HD               # 512 q columns per core
SCALE = 1.0 / float(np.sqrt(HD))

KC = H // 128              # 32 contraction chunks for projections
NQUART = 4                 # S split into 4 quarters of 512 for projections
QW = S // NQUART           # 512
NCH = NBLK // 2            # 8 attention chunks of 256 queries

F32 = mybir.dt.float32
BF16 = mybir.dt.bfloat16

# Opt-in profiling plumbing (off by default; harness never touches these).
TRACE = False
TRACE_KW: dict = {}
LAST_RESULTS = None


def _pair_js(i: int) -> list[int]:
    """Key blocks contributing to query pair i (blocks 2i, 2i+1)."""
    return sorted(set([0]) | set(range(max(0, 2 * i - 7), 2 * i + 2)))


def build_nc(rs: bool = True, repeat: int = 1):
    nc = bacc.Bacc(
        "TRN2", target_bir_lowering=False, debug=False, num_devices=N_CORES
    )
    hidT = nc.dram_tensor("hidT", [H, S], BF16, kind="ExternalInput").ap()
    wq = nc.dram_tensor("wq", [H, DQ], BF16, kind="ExternalInput").ap()
    wk = nc.dram_tensor("wk", [H, HD], BF16, kind="ExternalInput").ap()
    wv = nc.dram_tensor("wv", [H, HD], BF16, kind="ExternalInput").ap()
    wo = nc.dram_tensor("wo", [H, DQ], BF16, kind="ExternalInput").ap()
    cosF = nc.dram_tensor("cosF", [128, S], F32, kind="ExternalInput").ap()
    sinS = nc.dram_tensor("sinS", [128, S], F32, kind="ExternalInput").ap()
    tri = nc.dram_tensor("tri", [128, 128], BF16, kind="ExternalInput").ap()
    eye = nc.dram_tensor("eye", [128, 128], F32, kind="ExternalInput").ap()
    perm = nc.dram_tensor("perm", [128, 128], BF16, kind="ExternalInput").ap()
    onescol = nc.dram_tensor("onescol", [128, 1], BF16, kind="ExternalInput").ap()
    maskA = nc.dram_tensor("maskA", [128, 256], BF16, kind="ExternalInput").ap()
    out = nc.dram_tensor("out", [S, DQ], F32, kind="ExternalOutput").ap()

    with tile.TileContext(nc) as tc:
      for rep in range(repeat):
        with (
            tc.tile_pool(name=f"persist{rep}", bufs=1) as pp,
            tc.tile_pool(name=f"dram{rep}", bufs=1, space="DRAM") as dramp,
            tc.tile_pool(name=f"sp{rep}", bufs=1) as sp,
            tc.tile_pool(name=f"stream{rep}", bufs=4) as stp,
            tc.tile_pool(name=f"e_sb{rep}", bufs=4) as ep,
            tc.tile_pool(name=f"att_sb{rep}", bufs=2) as asb,
            tc.tile_pool(name=f"ev_sb{rep}", bufs=3) as evp,
        ):
            qTr = [
                [
                    pp.tile([128, QW], BF16, tag=f"qTr{h}_{nq}", name=f"qTr{h}_{nq}")
                    for nq in range(NQUART)
                ]
                for h in range(HQ)
            ]
            kTr = [
                pp.tile([128, QW], BF16, tag=f"kTr{nq}", name=f"kTr{nq}")
                for nq in range(NQUART)
            ]
            vNat = [
                pp.tile([128, QW], BF16, tag=f"vNat{nq}", name=f"vNat{nq}")
                for nq in range(NQUART)
            ]
            tri_sb = pp.tile([128, 128], BF16, tag="tri", name="tri_sb")
            maskA_sb = pp.tile([128, 256], BF16, tag="maskA", name="maskA_sb")
            eye_sb = pp.tile([128, 128], F32, tag="eye", name="eye_sb")
            perm_sb = pp.tile([128, 128], BF16, tag="perm", name="perm_sb")
            ones_sb = pp.tile([128, 1], BF16, tag="ones", name="ones_sb")
            # CC-stream warmup collective, issued first on gpsimd so the
            # ~50us NRT stream init overlaps the first projection quarter.
            if rs:
                warm_in = dramp.tile([16, 16], BF16, tag="warm_in", name="warm_in")
                warm_out = dramp.tile(
                    [128, 16], BF16, tag="warm_out", name="warm_out",
                    addr_space="Shared",
                )
                warm_src = pp.tile([16, 16], BF16, tag="warm_src", name="warm_src")
                nc.vector.memset(warm_src[:], 0.0)
                nc.gpsimd.dma_start(warm_in[:], warm_src[:])
                nc.gpsimd.collective_compute(
                    "AllGather",
                    mybir.AluOpType.bypass,
                    replica_groups=[list(range(N_CORES))],
                    ins=[warm_in.opt()],
                    outs=[warm_out.opt()],
                    unique_tensors="Yes",
                )

            # mask/table loads go on gpsimd: they're needed only from the
            # first attention chunk (~60us in), keeping the SP queue free
            # for weights and the ACT queue free for the quarter-0 hid
            # stream from t=0
            nc.gpsimd.dma_start(tri_sb[:], tri[:])
            nc.gpsimd.dma_start(maskA_sb[:], maskA[:])
            nc.gpsimd.dma_start(eye_sb[:], eye[:])
            nc.gpsimd.dma_start(perm_sb[:], perm[:])
            nc.gpsimd.dma_start(ones_sb[:], onescol[:])

            # weight prefetch: qkv weights issue on SP in consumption order
            # (interleaved per chunk, pacing the quarter-0 matmuls); wo on
            # gpsimd (needed only from the second slot on).
            wo_t = []
            for c in range(KC):
                tw = sp.tile([128, DQ], BF16, tag=f"woc{c}", name=f"woc{c}")
                nc.gpsimd.dma_start(tw[:], wo[c * 128 : (c + 1) * 128, :])
                wo_t.append(tw)
            wq_t, wk_t, wv_t = [], [], []
            for c in range(KC):
                crow = slice(c * 128, (c + 1) * 128)
                tq = sp.tile([128, DQ], BF16, tag=f"wqc{c}", name=f"wqc{c}")
                tk = sp.tile([128, HD], BF16, tag=f"wkc{c}", name=f"wkc{c}")
                tv = sp.tile([128, HD], BF16, tag=f"wvc{c}", name=f"wvc{c}")
                nc.sync.dma_start(tq[:], wq[crow, :])
                nc.sync.dma_start(tk[:], wk[crow, :])
                nc.sync.dma_start(tv[:], wv[crow, :])
                wq_t.append(tq)
                wk_t.append(tk)
                wv_t.append(tv)

            vT = sp.tile([128, S], F32, tag="vT", name="vT")
            ag_ins = [
                dramp.tile([DQ, 256], BF16, tag=f"agin{c}", name=f"agin{c}")
                for c in range(NCH)
            ]
            ag_outs = [
                dramp.tile(
                    [H, 256], BF16, tag=f"agout{c}", name=f"agout{c}",
                    addr_space="Shared",
                )
                for c in range(NCH)
            ]

            def attn_chunk(i: int, apool):
                q0 = i * 256
                js = _pair_js(i)
                L = len(js)
                for h in range(HQ):
                    e_t = ep.tile([128, L * 256], BF16, tag="e", name="e_t")
                    oT = apool.tile([128, 256], F32, tag="oT", name="oT", bufs=2)
                    sm = apool.tile([1, 256], F32, tag="sm", name="sm", bufs=1)

                    spans = []
                    for j in js:
                        left = (j == 0) or (j <= 2 * i <= j + 7)
                        right = (j == 0) or (j <= 2 * i + 1 <= j + 7)
                        qs = q0 if left else q0 + 128
                        qe = q0 + 256 if right else q0 + 128
                        spans.append((qs, qe))

                    def score(idx: int):
                        j = js[idx]
                        qs, qe = spans[idx]
                        w = qe - qs
                        ecols = slice(idx * 256, idx * 256 + w)
                        s_ps = apool.tile(
                            [128, 256], F32, tag="sps", name="s_ps", bufs=3
                        )
                        kq_, kc_ = j // 4, (j % 4) * 128
                        qq_ = qs // QW
                        nc.tensor.matmul(
                            s_ps[:, 0:w],
                            kTr[kq_][:, kc_ : kc_ + 128],
                            qTr[h][qq_][:, qs - qq_ * QW : qe - qq_ * QW],
                            start=True,
                            stop=True,
                        )
                        nc.scalar.activation(
                            e_t[:, ecols],
                            s_ps[:, 0:w],
                            mybir.ActivationFunctionType.Exp,
                            scale=SCALE,
                        )
                        if j == 2 * i:
                            nc.vector.tensor_mul(
                                e_t[:, ecols], e_t[:, ecols], maskA_sb[:]
                            )
                        elif j == 2 * i + 1:
                            nc.vector.tensor_mul(
                                e_t[:, ecols], e_t[:, ecols], tri_sb[:]
                            )

                    def av(idx: int):
                        j = js[idx]
                        qs, qe = spans[idx]
                        w = qe - qs
                        ecols = slice(idx * 256, idx * 256 + w)
                        st, sp_ = (idx == 0), (idx == L - 1)
                        nc.tensor.matmul(
                            oT[:, qs - q0 : qe - q0],
                            vNat[j // 4][:, (j % 4) * 128 : (j % 4 + 1) * 128],
                            e_t[:, ecols],
                            start=st,
                            stop=sp_,
                        )
                        nc.tensor.matmul(
                            sm[:, qs - q0 : qe - q0],
                            ones_sb[:],
                            e_t[:, ecols],
                            start=st,
                            stop=sp_,
                        )

                    score(0)
                    if L > 1:
                        score(1)
                    for idx in range(L):
                        if idx + 2 < L:
                            score(idx + 2)
                        av(idx)

                    r_sb = asb.tile([1, 256], F32, tag="r", name="r_sb")
                    nc.vector.reciprocal_approx_fast(r_sb[:], sm[:])
                    rb = asb.tile([128, 256], F32, tag="rb", name="rb")
                    nc.gpsimd.partition_broadcast(rb[:], r_sb[:])
                    at_c = asb.tile([128, 256], BF16, tag=f"at{h}", name=f"at{h}")
                    nc.vector.tensor_mul(at_c[:], oT[:], rb[:])
                    nc.sync.dma_start(
                        ag_ins[i][h * 128 : (h + 1) * 128, :], at_c[:]
                    )

                if rs:
                    nc.gpsimd.collective_compute(
                        "AllGather",
                        mybir.AluOpType.bypass,
                        replica_groups=[list(range(N_CORES))],
                        ins=[ag_ins[i].opt()],
                        outs=[ag_outs[i].opt()],
                        unique_tensors="Yes",
                    )
                else:
                    nc.sync.dma_start(ag_outs[i][0:DQ, :], ag_ins[i][:])

            def oproj_chunk(i: int, oppool):
                q0 = i * 256
                ps01 = [
                    oppool.tile(
                        [128, 512], F32, tag=f"op{sb}", name=f"op{sb}", bufs=1
                    )
                    for sb in range(2)
                ]
                # ag loads issue on gpsimd: they depend on collectives, and a
                # scheduler-hoisted one on the SP queue head-of-line blocks
                # the hid stream behind an in-flight AllGather
                for c in range(KC):
                    ag_sb = evp.tile(
                        [128, 256], BF16, tag="ag_sb", name="ag_sb", bufs=8
                    )
                    nc.gpsimd.dma_start(
                        ag_sb[:], ag_outs[i][c * 128 : (c + 1) * 128, :]
                    )
                    for sb in range(2):
                        nc.tensor.matmul(
                            ps01[sb][:],
                            ag_sb[:, sb * 128 : (sb + 1) * 128],
                            wo_t[c][:],
                            start=(c == 0),
                            stop=(c == KC - 1),
                        )
                for sb in range(2):
                    ev = evp.tile([128, 512], F32, tag="ev", name="ev")
                    nc.scalar.copy(ev[:], ps01[sb][:])
                    nc.gpsimd.dma_start(
                        out[q0 + sb * 128 : q0 + (sb + 1) * 128, :], ev[:]
                    )

            for nq in range(NQUART):
                ncols = slice(nq * QW, (nq + 1) * QW)
                with tc.tile_pool(
                    name=f"qps{rep}_{nq}", bufs=1, space="PSUM"
                ) as qpool:
                    ps_q = [
                        qpool.tile([128, QW], F32, tag=f"psq{h}", name=f"psq{h}")
                        for h in range(HQ)
                    ]
                    ps_k = qpool.tile([128, QW], F32, tag="psk", name="ps_k")
                    ps_v = qpool.tile([128, QW], F32, tag="psv", name="ps_v")
                    # quarter 0's hid stream issues on ACT so it doesn't sit
                    # behind the 96 weight DMAs on the SP queue
                    hid_eng = nc.scalar if nq == 0 else nc.sync
                    for c in range(KC):
                        crow = slice(c * 128, (c + 1) * 128)
                        hid_c = stp.tile([128, QW], BF16, tag="hid", name="hid_c")
                        hid_eng.dma_start(hid_c[:], hidT[crow, ncols])
                        st, sp_ = (c == 0), (c == KC - 1)
                        for h in range(HQ):
                            nc.tensor.matmul(
                                ps_q[h][:],
                                wq_t[c][:, h * HD : (h + 1) * HD],
                                hid_c[:],
                                start=st,
                                stop=sp_,
                            )
                        nc.tensor.matmul(
                            ps_k[:], wk_t[c][:], hid_c[:], start=st, stop=sp_
                        )
                        nc.tensor.matmul(
                            ps_v[:], wv_t[c][:], hid_c[:], start=st, stop=sp_
                        )

                    cos_sb = stp.tile([128, QW], F32, tag="cos", name="cos_sb", bufs=2)
                    sin_sb = stp.tile([128, QW], F32, tag="sin", name="sin_sb", bufs=2)
                    nc.sync.dma_start(cos_sb[:], cosF[:, ncols])
                    nc.sync.dma_start(sin_sb[:], sinS[:, ncols])

                    # Evacuate PSUM on ACT (bf16 raws) and RoPE on DVE,
                    # grouped per tensor with q0 first then k: the first
                    # attention chunk needs qTr[0] and the fresh kTr
                    # earliest. The rotate-half swap runs as a PE
                    # permutation matmul into PSUM (PE is idle here and it
                    # beats a DMA round trip by ~3us of latency).
                    rope_list = [(ps_q[0], qTr[0][nq], "q0"), (ps_k, kTr[nq], "k")]
                    rope_list += [
                        (ps_q[h], qTr[h][nq], f"q{h}") for h in range(1, HQ)
                    ]
                    with tc.tile_pool(
                        name=f"trp{rep}_{nq}", bufs=2, space="PSUM"
                    ) as trpool:
                        for ps_x, dstT, tag in rope_list:
                            raw = sp.tile(
                                [128, QW], BF16, tag=f"raw{tag}", name=f"raw{tag}"
                            )
                            nc.scalar.copy(raw[:], ps_x[:])
                            swp = trpool.tile([128, QW], F32, tag="tr", name="swp")
                            nc.tensor.matmul(
                                swp[:], perm_sb[:], raw[:], start=True, stop=True
                            )
                            t1 = sp.tile([128, QW], F32, tag=f"t1{tag}", name=f"t1{tag}")
                            nc.vector.tensor_mul(t1[:], raw[:], cos_sb[:])
                            t2 = sp.tile([128, QW], F32, tag=f"t2{tag}", name=f"t2{tag}")
                            nc.vector.tensor_mul(t2[:], swp[:], sin_sb[:])
                            nc.vector.tensor_add(dstT[:], t1[:], t2[:])
                        nc.scalar.copy(vT[:, ncols], ps_v[:])

                        # V natural blocks for this quarter (4 transposes)
                        for jb in range(nq * QW // 128, (nq + 1) * QW // 128):
                            bcols = slice(jb * 128, (jb + 1) * 128)
                            lcols = slice((jb % 4) * 128, (jb % 4 + 1) * 128)
                            tr = trpool.tile([128, QW], F32, tag="tr", name="tr")
                            nc.tensor.transpose(
                                tr[:, 0:128], vT[:, bcols], eye_sb[:]
                            )
                            nc.scalar.copy(vNat[nq][:, lcols], tr[:, 0:128])

                # attention chunks unlocked by this quarter; o_proj lags by
                # 2 chunks so each AllGather hides under subsequent PE work.
                # The last slot runs chunk 7 before 6 so AG7 is covered by
                # attn(6) + o_proj work instead of sticking out as a tail.
                # attention chunks unlocked by this quarter; o_proj lags by
                # 2 chunks so each AllGather hides under subsequent PE work.
                # The last slot runs chunk 7 before 6 so AG7 is covered by
                # attn(6) + o_proj work instead of sticking out as a tail.
                ca, cb = (2 * nq, 2 * nq + 1) if nq < 3 else (7, 6)
                with tc.tile_pool(
                    name=f"aps{rep}_{nq}", bufs=1, space="PSUM"
                ) as apool:
                    attn_chunk(ca, apool)
                    attn_chunk(cb, apool)
                    # both lagged o_projs after both attention chunks: all
                    # broadcasts (AG critical path) precede all ag loads on
                    # the gpsimd queue, and the loads' AllGathers are 1.5+
                    # slots old so they never head-of-line block anything
                    for lag in (2 * nq - 2, 2 * nq - 1):
                        if lag >= 0:
                            with tc.tile_pool(
                                name=f"ops{rep}_{nq}_{lag}", bufs=1, space="PSUM"
                            ) as oppool:
                                oproj_chunk(lag, oppool)

            with tc.tile_pool(
                name=f"ops{rep}_t6", bufs=1, space="PSUM"
            ) as oppool:
                oproj_chunk(NCH - 2, oppool)
            with tc.tile_pool(
                name=f"ops{rep}_t7", bufs=1, space="PSUM"
            ) as oppool:
                oproj_chunk(NCH - 1, oppool)
    nc.compile()
    return nc


@functools.lru_cache(maxsize=1)
def _cached_nc():
    return build_nc(rs=True)


def _tables():
    pos = np.arange(S, dtype=np.float64)
    inv = 1.0 / (ROPE_BASE ** (np.arange(0, HD, 2, dtype=np.float64) / HD))  # [64]
    f = inv[:, None] * pos[None, :]                   # [64, S]
    cos = np.cos(f).astype(np.float32)
    sin = np.sin(f).astype(np.float32)
    cosF = np.concatenate([cos, cos], axis=0)         # [128, S]
    sinS = np.concatenate([-sin, sin], axis=0)        # [128, S]
    k_idx = np.arange(128)[:, None]
    q_idx = np.arange(128)[None, :]
    tri = (k_idx <= q_idx).astype(np.float32)         # [k, q] causal in-block
    eye = np.eye(128, dtype=np.float32)
    maskA = np.concatenate([tri, np.ones((128, 128), np.float32)], axis=1)
    # rotate-half permutation: out[m] = in[(m+64) % 128] via P^T @ in
    permM = np.zeros((128, 128), np.float32)
    permM[(np.arange(128) + 64) % 128, np.arange(128)] = 1.0
    return cosF, sinS, tri, eye, maskA, permM


def _bf16(x: np.ndarray) -> np.ndarray:
    return np.ascontiguousarray(x).astype(ml_dtypes.bfloat16)


def kernel(hidden_states, wq, wk, wv, wo):
    nc = _cached_nc()
    hidT = _bf16(np.asarray(hidden_states, dtype=np.float32).reshape(S, H).T)
    cosF, sinS, tri, eye, maskA, permM = _tables()
    in_maps = []
    for c in range(N_CORES):
        in_maps.append(
            {
                "hidT": hidT,
                "wq": _bf16(wq[:, c * DQ : (c + 1) * DQ]),
                "wk": _bf16(wk[:, c * HD : (c + 1) * HD]),
                "wv": _bf16(wv[:, c * HD : (c + 1) * HD]),
                "wo": _bf16(wo[:, c * DQ : (c + 1) * DQ]),
                "cosF": cosF,
                "sinS": sinS,
                "tri": _bf16(tri),
                "eye": eye,
                "perm": _bf16(permM),
                "onescol": np.ones((128, 1), dtype=ml_dtypes.bfloat16),
                "maskA": _bf16(maskA),
            }
        )
    kw = dict(trace=True, **TRACE_KW) if TRACE else {}
    res = run_bass_kernel_spmd(nc, in_maps, core_ids=list(range(N_CORES)), **kw)
    global LAST_RESULTS
    LAST_RESULTS = res
    full = np.concatenate(
        [res.results[r]["out"] for r in range(N_CORES)], axis=1
    )
    return full.reshape(B, S, H)
